# revision 15
# baseline (speedup 1.0000x reference)
"""Trainium2 Bass kernel for per-sample dynamic (CDNA) depthwise 5x5 conv.

Computation (per sample b):
  k = relu(emb_flat @ W.T + b - 1e-5) + 1e-5        [225] -> [9, 25]
  k = k / k.sum(-1, keepdims=True)                  normalized 5x5 kernels
  out[k,c,h,w] = sum_{i,j} k[k,5i+j] * pad(rgb)[c,h+i,w+j]   [9,3,256,256]

Sharding: data-parallel over batch, 4 samples per core on 8 cores.

Conv-as-matmul mapping ("full-tap banded weights", K=90):
  Output rows are tiled HH=14 at a time (M = 14 rows x 9 kernels = 126,
  m = hh*9 + k, padded to 128 columns so bf16 LDWEIGHTS takes the
  fast-weight-load path). The contraction dim packs BOTH tap
  directions: q = j*18 + r with r an input row inside the tile's 18-row
  window and j the horizontal tap. lhsT[q, m] = kn[k, 5*(r-hh)+j]
  (banded in r-hh), rhs[q, col=(t,c,w)] = padded[14t+r, c, w+j]. One
  matmul per 512-column slab covers the whole 5x5 conv -- no PSUM
  accumulation chain, and one weight matrix per sample serves all 19
  row-tiles (the 4-row tail tile reads host-zeroed rhs rows).

  The pre-shifted rhs is built on the HOST (stg[b, 18j+r, t, c, w] =
  padded[b, 14t+r, c, w+j], bf16) so staging is one big contiguous DMA
  per sample. The normalized output accumulates in SBUF (bf16) in the
  native matmul layout [m=(hh,k), (t,c,w)] and is dumped to DRAM with
  ONE contiguous line-rate DMA per sample; the HOST permutes axes to
  [K,C,H,W] and upcasts to fp32 (pure layout transform -- all math and
  all output bytes still go through the device). PSUM evacuation
  alternates DVE / ScalarE (parallel on different PSUM banks).
  Rel-err ~6e-3, well under the 2e-2 gate.
"""

import sys
import numpy as np

try:
    import concourse  # noqa: F401
except ImportError:
    sys.path.insert(0, "/opt/trn_rl_repo")

KER = 5
NK = 9
SHIFT = 1e-5
B, C, H, W_IMG = 32, 3, 256, 256
PAD = KER // 2
HP = H + 2 * PAD  # 260
NCORES = 8
BL = B // NCORES  # 4 batches per core
FCIN = 8192
FCOUT = NK * KER * KER  # 225
HH = 14             # output rows per conv tile
M_FULL = NK * HH    # 126 live output columns
M_PAD = 128         # padded (FWL wants 128 weight columns)
KR = HH + KER - 1   # 18 input rows per tile window
KQ = KER * KR       # 90 contraction size (j, r)
NTILES = (H + HH - 1) // HH  # 19 (18 full + one 4-row tile)
H_LAST = H - (NTILES - 1) * HH  # 4
NCHUNK = FCIN // 128  # 64
NCOL = NTILES * C * W_IMG  # 14592 columns per sample
NSLAB = 512
STG_SPLIT = 2
HPAD = 274  # padded rows incl zero tail so 14*18+17 stays in range
M0, M1 = 128, FCOUT - 128  # fc output split (M0=128 enables FWL)

_CACHE = {}


def _build_nc():
    import concourse.bass as bass
    import concourse.bacc as bacc
    import concourse.mybir as mybir
    from concourse import tile
    from contextlib import ExitStack

    f32 = mybir.dt.float32
    bf16 = mybir.dt.bfloat16

    nc = bacc.Bacc("TRN2", target_bir_lowering=False, debug=False)

    # inputs (host-prepped layouts)
    embtp = nc.dram_tensor("embtp", [128, NCHUNK * BL], f32,
                           kind="ExternalInput").ap()
    wtp = nc.dram_tensor("wtp", [128, NCHUNK * FCOUT], bf16,
                         kind="ExternalInput").ap()
    biasm = nc.dram_tensor("biasm", [FCOUT, 1], f32,
                           kind="ExternalInput").ap()
    bones = nc.dram_tensor("bones", [FCOUT, NK], f32,
                           kind="ExternalInput").ap()
    stg = nc.dram_tensor("stg", [BL, KQ, NCOL], bf16,
                         kind="ExternalInput").ap()
    # raw output dump in matmul-native layout; host permutes to [K,C,H,W]
    out = nc.dram_tensor("out", [BL, M_FULL, NCOL], bf16,
                         kind="ExternalOutput").ap()

    # DRAM scratch
    knd = nc.dram_tensor("knd", [FCOUT, BL], f32)     # relu'd fc outputs
    zrd = nc.dram_tensor("zrd", [BL, M_FULL], f32)    # 1/Z at m=(hh,k)
    banded = nc.dram_tensor("banded", [BL, KER, KR, M_PAD], bf16)

    WSPLIT = 4
    CPS = NCHUNK // WSPLIT  # fc chunks per wt split

    with tile.TileContext(nc) as tc, ExitStack() as ctx:
        persist = ctx.enter_context(tc.tile_pool(name="persist", bufs=1))
        setup = ctx.enter_context(tc.tile_pool(name="setup", bufs=1))
        conv_in = ctx.enter_context(tc.tile_pool(name="conv_in", bufs=3))
        conv_out = ctx.enter_context(tc.tile_pool(name="conv_out", bufs=2))
        psum_fc = ctx.enter_context(
            tc.tile_pool(name="psum_fc", bufs=1, space="PSUM"))
        psum_conv = ctx.enter_context(
            tc.tile_pool(name="psum_conv", bufs=4, space="PSUM"))

        # ---- zero-init banded early (overlaps weight DMA) ----
        ztile = setup.tile([KR, BL * KER * M_PAD], bf16, tag="ztile")
        nc.vector.memset(ztile[:], 0.0)
        nc.gpsimd.dma_start(
            banded.ap().rearrange("b j r m -> r b j m"),
            ztile[:].rearrange("r (b j m) -> r b j m", b=BL, j=KER),
        )

        # ---- FC: kn[n, b] = W[n] . emb[b]  (fc-major output) ----
        wt_sb = []
        for s in range(WSPLIT):
            w_s = setup.tile([128, CPS * FCOUT], bf16, tag=f"wt{s}")
            eng = nc.sync if s % 2 == 0 else nc.scalar
            eng.dma_start(
                w_s[:], wtp[:, s * CPS * FCOUT:(s + 1) * CPS * FCOUT])
            wt_sb.append(w_s)
        embt_sb = setup.tile([128, NCHUNK * BL], f32, tag="embt")
        nc.scalar.dma_start(embt_sb[:], embtp)

        # ---- pre-stage conv rhs for b=0..2 (no deps; must be issued
        # BEFORE the band-build chain to avoid head-of-line blocking on
        # the HWDGE queues) ----
        nslabs = (NCOL + NSLAB - 1) // NSLAB  # 29 (28 full + 1 of 256)
        bounds = [0, (nslabs // 2) * NSLAB, NCOL]
        staged = {}
        def stage(b):
            parts = []
            for s in range(STG_SPLIT):
                lo, hi = bounds[s], bounds[s + 1]
                p_s = conv_in.tile([KQ, hi - lo], bf16, tag=f"staged{s}")
                eng = nc.sync if s % 2 == 0 else nc.scalar
                eng.dma_start(p_s[:], stg[b, :, lo:hi])
                parts.append(p_s)
            staged[b] = parts
        for b in range(min(3, BL)):
            stage(b)
        biasm_sb0 = setup.tile([M0, 1], f32, tag="biasm0")
        nc.sync.dma_start(biasm_sb0[:], biasm[0:M0])
        biasm_sb1 = setup.tile([M1, 1], f32, tag="biasm1")
        nc.sync.dma_start(biasm_sb1[:], biasm[M0:FCOUT])
        bones_sb0 = setup.tile([M0, NK], f32, tag="bones0")
        nc.sync.dma_start(bones_sb0[:], bones[0:M0])
        bones_sb1 = setup.tile([M1, NK], f32, tag="bones1")
        nc.sync.dma_start(bones_sb1[:], bones[M0:FCOUT])

        # fc matmuls: lhsT = wt chunk (bf16), rhs = emb chunk (cast bf16)
        embt_bf = setup.tile([128, NCHUNK * BL], bf16, tag="embt_bf")
        nc.vector.tensor_copy(embt_bf[:], embt_sb[:])
        knp0 = psum_fc.tile([M0, BL], f32, tag="knp0")
        knp1 = psum_fc.tile([M1, BL], f32, tag="knp1")
        for ci in range(NCHUNK):
            s, o = divmod(ci, CPS)
            rhs = embt_bf[:, ci * BL:(ci + 1) * BL]
            nc.tensor.matmul(
                knp0[:],
                lhsT=wt_sb[s][:, o * FCOUT:o * FCOUT + M0],
                rhs=rhs, start=(ci == 0), stop=(ci == NCHUNK - 1),
            )
            nc.tensor.matmul(
                knp1[:],
                lhsT=wt_sb[s][:, o * FCOUT + M0:(o + 1) * FCOUT],
                rhs=rhs, start=(ci == 0), stop=(ci == NCHUNK - 1),
            )

        # knr = relu(fc + bias - shift) + shift
        knr0 = setup.tile([M0, BL], f32, tag="knr0")
        nc.scalar.activation(
            knr0[:], knp0[:], mybir.ActivationFunctionType.Relu,
            bias=biasm_sb0[:])
        nc.vector.tensor_scalar_add(knr0[:], knr0[:], SHIFT)
        knr1 = setup.tile([M1, BL], f32, tag="knr1")
        nc.scalar.activation(
            knr1[:], knp1[:], mybir.ActivationFunctionType.Relu,
            bias=biasm_sb1[:])
        nc.vector.tensor_scalar_add(knr1[:], knr1[:], SHIFT)

        # Z[b, k] = sum_p knr[25k+p, b] via ones matmul; zr = 1/Z
        zps = psum_fc.tile([BL, NK], f32, tag="zps")
        nc.tensor.matmul(zps[:], lhsT=knr0[:], rhs=bones_sb0[:],
                         start=True, stop=False)
        nc.tensor.matmul(zps[:], lhsT=knr1[:], rhs=bones_sb1[:],
                         start=False, stop=True)
        zr = setup.tile([BL, NK], f32, tag="zr")
        nc.vector.reciprocal(zr[:], zps[:])
        # zrep[b, hh*9+k] = zr[b, k]; bounce via DRAM to get rv [126, 1]
        zrep = setup.tile([BL, M_FULL], f32, tag="zrep")
        nc.vector.tensor_copy(
            zrep[:].rearrange("b (hh k) -> b hh k", hh=HH),
            zr[:].unsqueeze(1).broadcast_to([BL, HH, NK]),
        )
        nc.sync.dma_start(zrd.ap(), zrep[:])
        rv = []
        for b in range(BL):
            rv_b = persist.tile([M_FULL, 1], f32, tag=f"rv{b}")
            nc.sync.dma_start(rv_b[:], zrd.ap()[b].unsqueeze(1))
            rv.append(rv_b)

        # ---- build banded lhsT via DRAM stride tricks ----
        nc.sync.dma_start(knd.ap()[0:M0], knr0[:])
        nc.sync.dma_start(knd.ap()[M0:FCOUT], knr1[:])
        # kn_kpre[(j d), (k b)] = knd[25k+5d+j, b]
        kn_kpre = setup.tile([KER * KER, NK * BL], f32, tag="kn_kpre")
        for j in range(KER):
            nc.sync.dma_start(
                kn_kpre[j * KER:(j + 1) * KER].rearrange(
                    "d (k b) -> d k b", k=NK),
                bass.AP(knd, j * BL,
                        [[KER * BL, KER], [KER * KER * BL, NK], [1, BL]]),
            )
        # kn_k[(j d), (b hh k)] (bf16, hh-replicated)
        kn_k = setup.tile([KER * KER, BL * HH * NK], bf16, tag="kn_k")
        nc.vector.tensor_copy(
            kn_k[:].rearrange("p (b hh k) -> p b hh k", b=BL, hh=HH),
            kn_kpre[:].rearrange("p (k b) -> p b k", k=NK)
            .unsqueeze(2).broadcast_to([KER * KER, BL, HH, NK]),
        )
        # scatter band: banded[b, j, d+hh, hh*9+k] = kn_k[(j d), (b hh k)]
        for b in range(BL):
            for j in range(KER):
                dst = bass.AP(
                    banded, (b * KER + j) * KR * M_PAD,
                    [[M_PAD, KER],         # d (input-row offset)
                     [M_PAD + NK, HH],     # hh (diagonal: r and m step)
                     [1, NK]],             # k
                )
                src = kn_k[j * KER:(j + 1) * KER,
                           b * HH * NK:(b + 1) * HH * NK].rearrange(
                    "d (hh k) -> d hh k", hh=HH)
                nc.sync.dma_start(dst, src)
        # load lhsT [90, 128] per sample
        lt = []
        for b in range(BL):
            lt_b = persist.tile([KQ, M_PAD], bf16, tag=f"lt{b}")
            nc.sync.dma_start(
                lt_b[:],
                banded.ap()[b].rearrange("j r m -> (j r) m"),
            )
            lt.append(lt_b)

        # ---- conv main loop ----
        for b in range(BL):
            parts = staged[b]
            osb = conv_out.tile([M_FULL, NCOL], bf16, tag="osb")
            for mi in range(nslabs):
                o = mi * NSLAB
                n = min(NSLAB, NCOL - o)
                s = 0 if o < bounds[1] else 1
                so = o - bounds[s]
                ps = psum_conv.tile([M_PAD, NSLAB], f32, tag="ps")
                nc.tensor.matmul(
                    ps[:, 0:n], lhsT=lt[b][:],
                    rhs=parts[s][:, so:so + n],
                    start=True, stop=True,
                )
                # evacuate+normalize live rows, alternating DVE / ScalarE
                if mi % 2 == 0:
                    nc.vector.tensor_scalar(
                        osb[:, o:o + n], ps[0:M_FULL, 0:n], rv[b][:], None,
                        op0=mybir.AluOpType.mult,
                    )
                else:
                    nc.scalar.activation(
                        osb[:, o:o + n], ps[0:M_FULL, 0:n],
                        mybir.ActivationFunctionType.Copy,
                        scale=rv[b][:],
                    )
            # stage the next sample's rhs now that a conv_in buffer frees
            if b + 3 < BL:
                stage(b + 3)
            # one contiguous line-rate dump per sample (SWDGE: keeps the
            # HWDGE queues free for staging)
            nc.gpsimd.dma_start(out[b], osb[:])
    nc.compile()
    return nc


def _host_prep(emb, rgb, W, b):
    import ml_dtypes
    bf16 = ml_dtypes.bfloat16

    emb = np.asarray(emb, dtype=np.float32)
    rgb = np.asarray(rgb, dtype=np.float32)
    W = np.asarray(W, dtype=np.float32)
    b = np.asarray(b, dtype=np.float32)

    embt = emb.reshape(B, FCIN).T  # [8192, 32]
    # wtp[p, ci, n] = W[n, ci*128+p]
    wtp = np.ascontiguousarray(
        W.T.reshape(NCHUNK, 128, FCOUT).transpose(1, 0, 2)).astype(bf16)
    wtp = wtp.reshape(128, NCHUNK * FCOUT)
    biasm = (b - SHIFT).reshape(FCOUT, 1).astype(np.float32).copy()
    bonesm = np.zeros((FCOUT, NK), dtype=np.float32)
    for k in range(NK):
        bonesm[k * KER * KER:(k + 1) * KER * KER, k] = 1.0

    # padded rgb, [b, h, c, w] with zero tail rows; bf16
    ph = np.zeros((B, HPAD, C, HP), dtype=bf16)
    ph[:, PAD:PAD + H, :, PAD:PAD + W_IMG] = rgb.transpose(0, 2, 1, 3)
    sb, sh, sc, sw = ph.strides
    stgv = np.lib.stride_tricks.as_strided(
        ph, shape=(B, KER, KR, NTILES, C, W_IMG),
        strides=(sb, sw, sh, HH * sh, sc, sw))
    stg = np.ascontiguousarray(stgv).reshape(B, KQ, NCOL)

    in_maps = []
    for core in range(NCORES):
        sl = slice(core * BL, (core + 1) * BL)
        in_maps.append({
            "embtp": np.ascontiguousarray(
                embt[:, sl].reshape(NCHUNK, 128, BL)
                .transpose(1, 0, 2)).astype(np.float32)
                .reshape(128, NCHUNK * BL),
            "wtp": wtp,
            "biasm": biasm,
            "bones": bonesm,
            "stg": stg[sl],
        })
    return in_maps


def _unpack(raw):
    """[BL, 126, NCOL] bf16 raw dump -> [BL, 9, 3, 256, 256] f32."""
    a = np.asarray(raw).reshape(BL, HH, NK, NTILES, C, W_IMG)
    a = a.transpose(0, 2, 4, 3, 1, 5)  # [b, k, c, t, hh, w]
    a = a.reshape(BL, NK, C, NTILES * HH, W_IMG)[:, :, :, :H, :]
    return np.ascontiguousarray(a).astype(np.float32)


def get_nc(rep=1):
    key = "nc"
    if key not in _CACHE:
        _CACHE[key] = _build_nc()
    return _CACHE[key]


def kernel(emb, rgb, W, b):
    from concourse.bass_utils import run_bass_kernel_spmd

    assert emb.shape == (B, 128, 8, 8) and rgb.shape == (B, C, H, W_IMG)
    nc = get_nc()
    in_maps = _host_prep(emb, rgb, W, b)
    res = run_bass_kernel_spmd(nc, in_maps, list(range(NCORES)))
    return np.concatenate([_unpack(r["out"]) for r in res.results], axis=0)


# revision 17
# speedup vs baseline: 1.1273x; 1.1273x over previous
"""Trainium2 Bass kernel for per-sample dynamic (CDNA) depthwise 5x5 conv.

Computation (per sample b):
  k = relu(emb_flat @ W.T + b - 1e-5) + 1e-5        [225] -> [9, 25]
  k = k / k.sum(-1, keepdims=True)                  normalized 5x5 kernels
  out[k,c,h,w] = sum_{i,j} k[k,5i+j] * pad(rgb)[c,h+i,w+j]   [9,3,256,256]

Sharding: data-parallel over batch, 4 samples per core on 8 cores.

Conv-as-matmul mapping ("full-tap banded weights", K=90):
  Output rows are tiled HH=14 at a time (M = 14 rows x 9 kernels = 126,
  m = hh*9 + k, padded to 128 columns so bf16 LDWEIGHTS takes the
  fast-weight-load path). The contraction dim packs BOTH tap
  directions: q = j*18 + r with r an input row inside the tile's 18-row
  window and j the horizontal tap. lhsT[q, m] = kn[k, 5*(r-hh)+j]
  (banded in r-hh), rhs[q, col=(t,c,w)] = padded[14t+r, c, w+j]. One
  matmul per 512-column slab covers the whole 5x5 conv -- no PSUM
  accumulation chain, and one weight matrix per sample serves all 19
  row-tiles (the 4-row tail tile reads host-zeroed rhs rows).

  The pre-shifted rhs is built on the HOST (stg[b, 18j+r, t, c, w] =
  padded[b, 14t+r, c, w+j], bf16) so staging is one big contiguous DMA
  per sample. The normalized output accumulates in SBUF (bf16) in the
  native matmul layout [m=(hh,k), (t,c,w)] and is dumped to DRAM with
  ONE contiguous line-rate DMA per sample; the HOST permutes axes to
  [K,C,H,W] and upcasts to fp32 (pure layout transform -- all math and
  all output bytes still go through the device). PSUM evacuation
  alternates DVE / ScalarE (parallel on different PSUM banks).
  Rel-err ~6e-3, well under the 2e-2 gate.
"""

import sys
import numpy as np

try:
    import concourse  # noqa: F401
except ImportError:
    sys.path.insert(0, "/opt/trn_rl_repo")

KER = 5
NK = 9
SHIFT = 1e-5
B, C, H, W_IMG = 32, 3, 256, 256
PAD = KER // 2
HP = H + 2 * PAD  # 260
NCORES = 8
BL = B // NCORES  # 4 batches per core
FCIN = 8192
FCOUT = NK * KER * KER  # 225
HH = 14             # output rows per conv tile
M_FULL = NK * HH    # 126 live output columns
M_PAD = 128         # padded (FWL wants 128 weight columns)
KR = HH + KER - 1   # 18 input rows per tile window
KQ = KER * KR       # 90 contraction size (j, r)
NTILES = (H + HH - 1) // HH  # 19 (18 full + one 4-row tile)
H_LAST = H - (NTILES - 1) * HH  # 4
NCHUNK = FCIN // 128  # 64
NCOL = NTILES * C * W_IMG  # 14592 columns per sample
NSLAB = 512
STG_SPLIT = 2
HPAD = 274  # padded rows incl zero tail so 14*18+17 stays in range
M0, M1 = 128, FCOUT - 128  # fc output split (M0=128 enables FWL)

_CACHE = {}


def _build_nc():
    import concourse.bass as bass
    import concourse.bacc as bacc
    import concourse.mybir as mybir
    from concourse import tile
    from contextlib import ExitStack

    f32 = mybir.dt.float32
    bf16 = mybir.dt.bfloat16

    nc = bacc.Bacc("TRN2", target_bir_lowering=False, debug=False)

    # inputs (host-prepped layouts)
    embtp = nc.dram_tensor("embtp", [128, NCHUNK * BL], f32,
                           kind="ExternalInput").ap()
    wtp = nc.dram_tensor("wtp", [128, NCHUNK * FCOUT], bf16,
                         kind="ExternalInput").ap()
    biasm = nc.dram_tensor("biasm", [FCOUT, 1], f32,
                           kind="ExternalInput").ap()
    bones = nc.dram_tensor("bones", [FCOUT, NK], f32,
                           kind="ExternalInput").ap()
    stg = nc.dram_tensor("stg", [BL, KQ, NCOL], bf16,
                         kind="ExternalInput").ap()
    # raw output dump in matmul-native layout; host permutes to [K,C,H,W]
    out = nc.dram_tensor("out", [BL, M_FULL, NCOL], bf16,
                         kind="ExternalOutput").ap()

    # DRAM scratch
    knd = nc.dram_tensor("knd", [FCOUT, BL], f32)     # relu'd fc outputs
    zrd = nc.dram_tensor("zrd", [BL, M_FULL], f32)    # 1/Z at m=(hh,k)
    banded = nc.dram_tensor("banded", [BL, KER, KR, M_PAD], bf16)

    WSPLIT = 4
    CPS = NCHUNK // WSPLIT  # fc chunks per wt split

    with tile.TileContext(nc) as tc, ExitStack() as ctx:
        persist = ctx.enter_context(tc.tile_pool(name="persist", bufs=1))
        setup = ctx.enter_context(tc.tile_pool(name="setup", bufs=1))
        conv_in = ctx.enter_context(tc.tile_pool(name="conv_in", bufs=3))
        conv_out = ctx.enter_context(tc.tile_pool(name="conv_out", bufs=2))
        psum_fc = ctx.enter_context(
            tc.tile_pool(name="psum_fc", bufs=1, space="PSUM"))
        psum_conv = ctx.enter_context(
            tc.tile_pool(name="psum_conv", bufs=4, space="PSUM"))

        # ---- zero-init banded early (overlaps weight DMA) ----
        ztile = setup.tile([KR, BL * KER * M_PAD], bf16, tag="ztile")
        nc.vector.memset(ztile[:], 0.0)
        nc.gpsimd.dma_start(
            banded.ap().rearrange("b j r m -> r b j m"),
            ztile[:].rearrange("r (b j m) -> r b j m", b=BL, j=KER),
        )

        # ---- FC: kn[n, b] = W[n] . emb[b]  (fc-major output) ----
        # queue plan: sync ring = wt + latency-critical band chain;
        # scalar ring = small fc inputs then bulk staging; gpsimd/SWDGE =
        # banded-zero + output dumps.
        wt_sb = []
        for s in range(WSPLIT):
            w_s = setup.tile([128, CPS * FCOUT], bf16, tag=f"wt{s}")
            nc.sync.dma_start(
                w_s[:], wtp[:, s * CPS * FCOUT:(s + 1) * CPS * FCOUT])
            wt_sb.append(w_s)
        embt_sb = setup.tile([128, NCHUNK * BL], f32, tag="embt")
        nc.scalar.dma_start(embt_sb[:], embtp)
        biasm_sb0 = setup.tile([M0, 1], f32, tag="biasm0")
        nc.scalar.dma_start(biasm_sb0[:], biasm[0:M0])
        biasm_sb1 = setup.tile([M1, 1], f32, tag="biasm1")
        nc.scalar.dma_start(biasm_sb1[:], biasm[M0:FCOUT])
        bones_sb0 = setup.tile([M0, NK], f32, tag="bones0")
        nc.scalar.dma_start(bones_sb0[:], bones[0:M0])
        bones_sb1 = setup.tile([M1, NK], f32, tag="bones1")
        nc.scalar.dma_start(bones_sb1[:], bones[M0:FCOUT])

        # ---- pre-stage conv rhs for b=0..2 (no deps; issued ahead of
        # dependent DMAs to avoid head-of-line blocking on the rings) ----
        nslabs = (NCOL + NSLAB - 1) // NSLAB  # 29 (28 full + 1 of 256)
        bounds = [0, (nslabs // 2) * NSLAB, NCOL]
        staged = {}
        def stage(b):
            parts = []
            for s in range(STG_SPLIT):
                lo, hi = bounds[s], bounds[s + 1]
                p_s = conv_in.tile([KQ, hi - lo], bf16, tag=f"staged{s}")
                nc.scalar.dma_start(p_s[:], stg[b, :, lo:hi])
                parts.append(p_s)
            staged[b] = parts
        for b in range(min(3, BL)):
            stage(b)

        # fc matmuls: lhsT = wt chunk (bf16), rhs = emb chunk (cast bf16)
        embt_bf = setup.tile([128, NCHUNK * BL], bf16, tag="embt_bf")
        nc.vector.tensor_copy(embt_bf[:], embt_sb[:])
        knp0 = psum_fc.tile([M0, BL], f32, tag="knp0")
        knp1 = psum_fc.tile([M1, BL], f32, tag="knp1")
        for ci in range(NCHUNK):
            s, o = divmod(ci, CPS)
            rhs = embt_bf[:, ci * BL:(ci + 1) * BL]
            nc.tensor.matmul(
                knp0[:],
                lhsT=wt_sb[s][:, o * FCOUT:o * FCOUT + M0],
                rhs=rhs, start=(ci == 0), stop=(ci == NCHUNK - 1),
            )
            nc.tensor.matmul(
                knp1[:],
                lhsT=wt_sb[s][:, o * FCOUT + M0:(o + 1) * FCOUT],
                rhs=rhs, start=(ci == 0), stop=(ci == NCHUNK - 1),
            )

        # knr = relu(fc + bias - shift) + shift
        knr0 = setup.tile([M0, BL], f32, tag="knr0")
        nc.scalar.activation(
            knr0[:], knp0[:], mybir.ActivationFunctionType.Relu,
            bias=biasm_sb0[:])
        nc.vector.tensor_scalar_add(knr0[:], knr0[:], SHIFT)
        knr1 = setup.tile([M1, BL], f32, tag="knr1")
        nc.scalar.activation(
            knr1[:], knp1[:], mybir.ActivationFunctionType.Relu,
            bias=biasm_sb1[:])
        nc.vector.tensor_scalar_add(knr1[:], knr1[:], SHIFT)

        # Z[b, k] = sum_p knr[25k+p, b] via ones matmul; zr = 1/Z
        zps = psum_fc.tile([BL, NK], f32, tag="zps")
        nc.tensor.matmul(zps[:], lhsT=knr0[:], rhs=bones_sb0[:],
                         start=True, stop=False)
        nc.tensor.matmul(zps[:], lhsT=knr1[:], rhs=bones_sb1[:],
                         start=False, stop=True)
        zr = setup.tile([BL, NK], f32, tag="zr")
        nc.vector.reciprocal(zr[:], zps[:])
        # zrep[b, hh*9+k] = zr[b, k]; bounce via DRAM to get rv [126, 1]
        zrep = setup.tile([BL, M_FULL], f32, tag="zrep")
        nc.vector.tensor_copy(
            zrep[:].rearrange("b (hh k) -> b hh k", hh=HH),
            zr[:].unsqueeze(1).broadcast_to([BL, HH, NK]),
        )
        nc.sync.dma_start(zrd.ap(), zrep[:])
        rv = []
        for b in range(BL):
            rv_b = persist.tile([M_FULL, 1], f32, tag=f"rv{b}")
            nc.sync.dma_start(rv_b[:], zrd.ap()[b].unsqueeze(1))
            rv.append(rv_b)

        # ---- build banded lhsT via DRAM stride tricks ----
        nc.sync.dma_start(knd.ap()[0:M0], knr0[:])
        nc.sync.dma_start(knd.ap()[M0:FCOUT], knr1[:])
        # kn_kpre[(j d), (k b)] = knd[25k+5d+j, b]
        kn_kpre = setup.tile([KER * KER, NK * BL], f32, tag="kn_kpre")
        for j in range(KER):
            nc.sync.dma_start(
                kn_kpre[j * KER:(j + 1) * KER].rearrange(
                    "d (k b) -> d k b", k=NK),
                bass.AP(knd, j * BL,
                        [[KER * BL, KER], [KER * KER * BL, NK], [1, BL]]),
            )
        # kn_k[(j d), (b hh k)] (bf16, hh-replicated)
        kn_k = setup.tile([KER * KER, BL * HH * NK], bf16, tag="kn_k")
        nc.vector.tensor_copy(
            kn_k[:].rearrange("p (b hh k) -> p b hh k", b=BL, hh=HH),
            kn_kpre[:].rearrange("p (k b) -> p b k", k=NK)
            .unsqueeze(2).broadcast_to([KER * KER, BL, HH, NK]),
        )
        # scatter band: banded[b, j, d+hh, hh*9+k] = kn_k[(j d), (b hh k)]
        for b in range(BL):
            for j in range(KER):
                dst = bass.AP(
                    banded, (b * KER + j) * KR * M_PAD,
                    [[M_PAD, KER],         # d (input-row offset)
                     [M_PAD + NK, HH],     # hh (diagonal: r and m step)
                     [1, NK]],             # k
                )
                src = kn_k[j * KER:(j + 1) * KER,
                           b * HH * NK:(b + 1) * HH * NK].rearrange(
                    "d (hh k) -> d hh k", hh=HH)
                nc.sync.dma_start(dst, src)
        # load lhsT [90, 128] per sample
        lt = []
        for b in range(BL):
            lt_b = persist.tile([KQ, M_PAD], bf16, tag=f"lt{b}")
            nc.sync.dma_start(
                lt_b[:],
                banded.ap()[b].rearrange("j r m -> (j r) m"),
            )
            lt.append(lt_b)

        # ---- conv main loop ----
        for b in range(BL):
            parts = staged[b]
            osb = conv_out.tile([M_FULL, NCOL], bf16, tag="osb")
            for mi in range(nslabs):
                o = mi * NSLAB
                n = min(NSLAB, NCOL - o)
                s = 0 if o < bounds[1] else 1
                so = o - bounds[s]
                ps = psum_conv.tile([M_PAD, NSLAB], f32, tag="ps")
                nc.tensor.matmul(
                    ps[:, 0:n], lhsT=lt[b][:],
                    rhs=parts[s][:, so:so + n],
                    start=True, stop=True,
                )
                # evacuate+normalize live rows, alternating DVE / ScalarE
                if mi % 2 == 0:
                    nc.vector.tensor_scalar(
                        osb[:, o:o + n], ps[0:M_FULL, 0:n], rv[b][:], None,
                        op0=mybir.AluOpType.mult,
                    )
                else:
                    nc.scalar.activation(
                        osb[:, o:o + n], ps[0:M_FULL, 0:n],
                        mybir.ActivationFunctionType.Copy,
                        scale=rv[b][:],
                    )
            # stage the next sample's rhs now that a conv_in buffer frees
            if b + 3 < BL:
                stage(b + 3)
            # contiguous line-rate dump per sample, split in two so the
            # first half drains while the second half still evacuates
            # (SWDGE: keeps the HWDGE rings free for staging)
            nc.gpsimd.dma_start(out[b, :, 0:bounds[1]],
                                osb[:, 0:bounds[1]])
            nc.gpsimd.dma_start(out[b, :, bounds[1]:NCOL],
                                osb[:, bounds[1]:NCOL])
    nc.compile()
    return nc


def _host_prep(emb, rgb, W, b):
    import ml_dtypes
    bf16 = ml_dtypes.bfloat16

    emb = np.asarray(emb, dtype=np.float32)
    rgb = np.asarray(rgb, dtype=np.float32)
    W = np.asarray(W, dtype=np.float32)
    b = np.asarray(b, dtype=np.float32)

    embt = emb.reshape(B, FCIN).T  # [8192, 32]
    # wtp[p, ci, n] = W[n, ci*128+p]
    wtp = np.ascontiguousarray(
        W.T.reshape(NCHUNK, 128, FCOUT).transpose(1, 0, 2)).astype(bf16)
    wtp = wtp.reshape(128, NCHUNK * FCOUT)
    biasm = (b - SHIFT).reshape(FCOUT, 1).astype(np.float32).copy()
    bonesm = np.zeros((FCOUT, NK), dtype=np.float32)
    for k in range(NK):
        bonesm[k * KER * KER:(k + 1) * KER * KER, k] = 1.0

    # padded rgb, [b, h, c, w] with zero tail rows; bf16
    ph = np.zeros((B, HPAD, C, HP), dtype=bf16)
    ph[:, PAD:PAD + H, :, PAD:PAD + W_IMG] = rgb.transpose(0, 2, 1, 3)
    sb, sh, sc, sw = ph.strides
    stgv = np.lib.stride_tricks.as_strided(
        ph, shape=(B, KER, KR, NTILES, C, W_IMG),
        strides=(sb, sw, sh, HH * sh, sc, sw))
    stg = np.ascontiguousarray(stgv).reshape(B, KQ, NCOL)

    in_maps = []
    for core in range(NCORES):
        sl = slice(core * BL, (core + 1) * BL)
        in_maps.append({
            "embtp": np.ascontiguousarray(
                embt[:, sl].reshape(NCHUNK, 128, BL)
                .transpose(1, 0, 2)).astype(np.float32)
                .reshape(128, NCHUNK * BL),
            "wtp": wtp,
            "biasm": biasm,
            "bones": bonesm,
            "stg": stg[sl],
        })
    return in_maps


def _unpack(raw):
    """[BL, 126, NCOL] bf16 raw dump -> [BL, 9, 3, 256, 256] f32."""
    a = np.asarray(raw).reshape(BL, HH, NK, NTILES, C, W_IMG)
    a = a.transpose(0, 2, 4, 3, 1, 5)  # [b, k, c, t, hh, w]
    a = a.reshape(BL, NK, C, NTILES * HH, W_IMG)[:, :, :, :H, :]
    return np.ascontiguousarray(a).astype(np.float32)


def get_nc(rep=1):
    key = "nc"
    if key not in _CACHE:
        _CACHE[key] = _build_nc()
    return _CACHE[key]


def kernel(emb, rgb, W, b):
    from concourse.bass_utils import run_bass_kernel_spmd

    assert emb.shape == (B, 128, 8, 8) and rgb.shape == (B, C, H, W_IMG)
    nc = get_nc()
    in_maps = _host_prep(emb, rgb, W, b)
    res = run_bass_kernel_spmd(nc, in_maps, list(range(NCORES)))
    return np.concatenate([_unpack(r["out"]) for r in res.results], axis=0)


# revision 20
# speedup vs baseline: 1.2068x; 1.0705x over previous
"""Trainium2 Bass kernel for per-sample dynamic (CDNA) depthwise 5x5 conv.

Computation (per sample b):
  k = relu(emb_flat @ W.T + b - 1e-5) + 1e-5        [225] -> [9, 25]
  k = k / k.sum(-1, keepdims=True)                  normalized 5x5 kernels
  out[k,c,h,w] = sum_{i,j} k[k,5i+j] * pad(rgb)[c,h+i,w+j]   [9,3,256,256]

Sharding: data-parallel over batch, 4 samples per core on 8 cores.

Conv-as-matmul mapping ("full-tap banded weights", K=90):
  Output rows are tiled HH=14 at a time (M = 14 rows x 9 kernels = 126,
  m = hh*9 + k, padded to 128 columns so bf16 LDWEIGHTS takes the
  fast-weight-load path). The contraction dim packs BOTH tap
  directions: q = j*18 + r with r an input row inside the tile's 18-row
  window and j the horizontal tap. lhsT[q, m] = kn[k, 5*(r-hh)+j]
  (banded in r-hh), rhs[q, col=(t,c,w)] = padded[14t+r, c, w+j]. One
  matmul per 512-column slab covers the whole 5x5 conv -- no PSUM
  accumulation chain, and one weight matrix per sample serves all 19
  row-tiles (the 4-row tail tile reads host-zeroed rhs rows).

  The pre-shifted rhs is built on the HOST (stg[b, 18j+r, t, c, w] =
  padded[b, 14t+r, c, w+j], bf16) so staging is one big contiguous DMA
  per sample. The normalized output accumulates in SBUF (bf16) in the
  native matmul layout [m=(hh,k), (t,c,w)] and is dumped to DRAM with
  ONE contiguous line-rate DMA per sample; the HOST permutes axes to
  [K,C,H,W] and upcasts to fp32 (pure layout transform -- all math and
  all output bytes still go through the device). PSUM evacuation
  alternates DVE / ScalarE (parallel on different PSUM banks).
  Rel-err ~6e-3, well under the 2e-2 gate.
"""

import sys
import numpy as np

try:
    import concourse  # noqa: F401
except ImportError:
    sys.path.insert(0, "/opt/trn_rl_repo")

KER = 5
NK = 9
SHIFT = 1e-5
B, C, H, W_IMG = 32, 3, 256, 256
PAD = KER // 2
HP = H + 2 * PAD  # 260
NCORES = 8
BL = B // NCORES  # 4 batches per core
FCIN = 8192
FCOUT = NK * KER * KER  # 225
HH = 14             # output rows per conv tile
M_FULL = NK * HH    # 126 live output columns
M_PAD = 128         # padded (FWL wants 128 weight columns)
KR = HH + KER - 1   # 18 input rows per tile window
KQ = KER * KR       # 90 contraction size (j, r)
NTILES = (H + HH - 1) // HH  # 19 (18 full + one 4-row tile)
H_LAST = H - (NTILES - 1) * HH  # 4
NCHUNK = FCIN // 128  # 64
NCOL = NTILES * C * W_IMG  # 14592 columns per sample
NSLAB = 512
STG_SPLIT = 2
HPAD = 274  # padded rows incl zero tail so 14*18+17 stays in range
M0, M1 = 128, FCOUT - 128  # fc output split (M0=128 enables FWL)

_CACHE = {}


def _build_nc():
    import concourse.bass as bass
    import concourse.bacc as bacc
    import concourse.mybir as mybir
    from concourse import tile
    from contextlib import ExitStack

    f32 = mybir.dt.float32
    bf16 = mybir.dt.bfloat16

    nc = bacc.Bacc("TRN2", target_bir_lowering=False, debug=False)

    # inputs (host-prepped layouts)
    embtp = nc.dram_tensor("embtp", [128, NCHUNK * BL], f32,
                           kind="ExternalInput").ap()
    wtp = nc.dram_tensor("wtp", [128, NCHUNK * FCOUT], bf16,
                         kind="ExternalInput").ap()
    biasm = nc.dram_tensor("biasm", [FCOUT, 1], f32,
                           kind="ExternalInput").ap()
    bones = nc.dram_tensor("bones", [FCOUT, NK], f32,
                           kind="ExternalInput").ap()
    stg = nc.dram_tensor("stg", [BL, KQ, NCOL], bf16,
                         kind="ExternalInput").ap()
    # raw output dump in matmul-native layout; host permutes to [K,C,H,W]
    out = nc.dram_tensor("out", [BL, M_FULL, NCOL], bf16,
                         kind="ExternalOutput").ap()

    # DRAM scratch
    knd = nc.dram_tensor("knd", [FCOUT, BL], f32)     # relu'd fc outputs
    zrd = nc.dram_tensor("zrd", [BL, M_FULL], f32)    # 1/Z at m=(hh,k)
    banded = nc.dram_tensor("banded", [BL, KER, KR, M_PAD], bf16)

    WSPLIT = 4
    CPS = NCHUNK // WSPLIT  # fc chunks per wt split

    with tile.TileContext(nc) as tc, ExitStack() as ctx:
        persist = ctx.enter_context(tc.tile_pool(name="persist", bufs=1))
        setup = ctx.enter_context(tc.tile_pool(name="setup", bufs=1))
        conv_in = ctx.enter_context(tc.tile_pool(name="conv_in", bufs=3))
        conv_out = ctx.enter_context(tc.tile_pool(name="conv_out", bufs=2))
        psum_fc = ctx.enter_context(
            tc.tile_pool(name="psum_fc", bufs=1, space="PSUM"))
        psum_conv = ctx.enter_context(
            tc.tile_pool(name="psum_conv", bufs=4, space="PSUM"))

        # ---- zero-init banded early (overlaps weight DMA) ----
        ztile = setup.tile([KR, BL * KER * M_PAD], bf16, tag="ztile")
        nc.vector.memset(ztile[:], 0.0)
        nc.gpsimd.dma_start(
            banded.ap().rearrange("b j r m -> r b j m"),
            ztile[:].rearrange("r (b j m) -> r b j m", b=BL, j=KER),
        )
        # round-robin helper for latency-critical small DMAs: alternate
        # between the two HWDGE rings so the ~0.7us HBM round trips overlap
        _rr = [0]
        def small_dma(dst, src):
            eng = nc.sync if _rr[0] % 2 == 0 else nc.scalar
            _rr[0] += 1
            eng.dma_start(dst, src)

        # ---- FC: kn[n, b] = W[n] . emb[b]  (fc-major output) ----
        # queue plan: sync ring = wt + latency-critical band chain;
        # scalar ring = small fc inputs then bulk staging; gpsimd/SWDGE =
        # banded-zero + output dumps.
        wt_sb = []
        for s in range(WSPLIT):
            w_s = setup.tile([128, CPS * FCOUT], bf16, tag=f"wt{s}")
            nc.sync.dma_start(
                w_s[:], wtp[:, s * CPS * FCOUT:(s + 1) * CPS * FCOUT])
            wt_sb.append(w_s)
        embt_sb = setup.tile([128, NCHUNK * BL], f32, tag="embt")
        nc.scalar.dma_start(embt_sb[:], embtp)
        biasm_sb0 = setup.tile([M0, 1], f32, tag="biasm0")
        nc.scalar.dma_start(biasm_sb0[:], biasm[0:M0])
        biasm_sb1 = setup.tile([M1, 1], f32, tag="biasm1")
        nc.scalar.dma_start(biasm_sb1[:], biasm[M0:FCOUT])
        bones_sb0 = setup.tile([M0, NK], f32, tag="bones0")
        nc.scalar.dma_start(bones_sb0[:], bones[0:M0])
        bones_sb1 = setup.tile([M1, NK], f32, tag="bones1")
        nc.scalar.dma_start(bones_sb1[:], bones[M0:FCOUT])

        # ---- pre-stage conv rhs for b=0..2 on the SWDGE queue (bulk
        # traffic; keeps both HWDGE rings free for latency-critical DMAs
        # and the scalar engine free for activations) ----
        nslabs = (NCOL + NSLAB - 1) // NSLAB  # 29 (28 full + 1 of 256)
        bounds = [0, (nslabs // 2) * NSLAB, NCOL]
        staged = {}
        def stage(b):
            parts = []
            for s in range(STG_SPLIT):
                lo, hi = bounds[s], bounds[s + 1]
                p_s = conv_in.tile([KQ, hi - lo], bf16, tag=f"staged{s}")
                nc.gpsimd.dma_start(p_s[:], stg[b, :, lo:hi])
                parts.append(p_s)
            staged[b] = parts
        for b in range(min(3, BL)):
            stage(b)

        # fc matmuls: lhsT = wt chunk (bf16), rhs = emb chunk (cast bf16)
        embt_bf = setup.tile([128, NCHUNK * BL], bf16, tag="embt_bf")
        nc.vector.tensor_copy(embt_bf[:], embt_sb[:])
        knp0 = psum_fc.tile([M0, BL], f32, tag="knp0")
        knp1 = psum_fc.tile([M1, BL], f32, tag="knp1")
        for ci in range(NCHUNK):
            s, o = divmod(ci, CPS)
            rhs = embt_bf[:, ci * BL:(ci + 1) * BL]
            nc.tensor.matmul(
                knp0[:],
                lhsT=wt_sb[s][:, o * FCOUT:o * FCOUT + M0],
                rhs=rhs, start=(ci == 0), stop=(ci == NCHUNK - 1),
            )
            nc.tensor.matmul(
                knp1[:],
                lhsT=wt_sb[s][:, o * FCOUT + M0:(o + 1) * FCOUT],
                rhs=rhs, start=(ci == 0), stop=(ci == NCHUNK - 1),
            )

        # knr = relu(fc + bias - shift) + shift
        knr0 = setup.tile([M0, BL], f32, tag="knr0")
        nc.scalar.activation(
            knr0[:], knp0[:], mybir.ActivationFunctionType.Relu,
            bias=biasm_sb0[:])
        nc.vector.tensor_scalar_add(knr0[:], knr0[:], SHIFT)
        knr1 = setup.tile([M1, BL], f32, tag="knr1")
        nc.scalar.activation(
            knr1[:], knp1[:], mybir.ActivationFunctionType.Relu,
            bias=biasm_sb1[:])
        nc.vector.tensor_scalar_add(knr1[:], knr1[:], SHIFT)

        # Z[b, k] = sum_p knr[25k+p, b] via ones matmul; zr = 1/Z
        zps = psum_fc.tile([BL, NK], f32, tag="zps")
        nc.tensor.matmul(zps[:], lhsT=knr0[:], rhs=bones_sb0[:],
                         start=True, stop=False)
        nc.tensor.matmul(zps[:], lhsT=knr1[:], rhs=bones_sb1[:],
                         start=False, stop=True)
        zr = setup.tile([BL, NK], f32, tag="zr")
        nc.vector.reciprocal(zr[:], zps[:])

        # ---- build banded lhsT via DRAM stride tricks (latency-critical
        # chain: knd -> kn_kpre -> kn_k -> scatter -> lt; small DMAs
        # round-robined over both HWDGE rings) ----
        small_dma(knd.ap()[0:M0], knr0[:])
        small_dma(knd.ap()[M0:FCOUT], knr1[:])
        # kn_kpre[(j d), (k b)] = knd[25k+5d+j, b]
        kn_kpre = setup.tile([KER * KER, NK * BL], f32, tag="kn_kpre")
        for j in range(KER):
            small_dma(
                kn_kpre[j * KER:(j + 1) * KER].rearrange(
                    "d (k b) -> d k b", k=NK),
                bass.AP(knd, j * BL,
                        [[KER * BL, KER], [KER * KER * BL, NK], [1, BL]]),
            )
        # zrep[b, hh*9+k] = zr[b, k]; bounce via DRAM to get rv [126, 1]
        zrep = setup.tile([BL, M_FULL], f32, tag="zrep")
        nc.vector.tensor_copy(
            zrep[:].rearrange("b (hh k) -> b hh k", hh=HH),
            zr[:].unsqueeze(1).broadcast_to([BL, HH, NK]),
        )
        small_dma(zrd.ap(), zrep[:])
        rv = []
        for b in range(BL):
            rv_b = persist.tile([M_FULL, 1], f32, tag=f"rv{b}")
            small_dma(rv_b[:], zrd.ap()[b].unsqueeze(1))
            rv.append(rv_b)
        # kn_k[(j d), (b hh k)] (bf16, hh-replicated)
        kn_k = setup.tile([KER * KER, BL * HH * NK], bf16, tag="kn_k")
        nc.vector.tensor_copy(
            kn_k[:].rearrange("p (b hh k) -> p b hh k", b=BL, hh=HH),
            kn_kpre[:].rearrange("p (k b) -> p b k", k=NK)
            .unsqueeze(2).broadcast_to([KER * KER, BL, HH, NK]),
        )
        # scatter band: banded[b, j, d+hh, hh*9+k] = kn_k[(j d), (b hh k)]
        for b in range(BL):
            for j in range(KER):
                dst = bass.AP(
                    banded, (b * KER + j) * KR * M_PAD,
                    [[M_PAD, KER],         # d (input-row offset)
                     [M_PAD + NK, HH],     # hh (diagonal: r and m step)
                     [1, NK]],             # k
                )
                src = kn_k[j * KER:(j + 1) * KER,
                           b * HH * NK:(b + 1) * HH * NK].rearrange(
                    "d (hh k) -> d hh k", hh=HH)
                small_dma(dst, src)
        # load lhsT [90, 128] per sample
        lt = []
        for b in range(BL):
            lt_b = persist.tile([KQ, M_PAD], bf16, tag=f"lt{b}")
            small_dma(
                lt_b[:],
                banded.ap()[b].rearrange("j r m -> (j r) m"),
            )
            lt.append(lt_b)

        # ---- conv main loop ----
        for b in range(BL):
            parts = staged[b]
            osb = conv_out.tile([M_FULL, NCOL], bf16, tag="osb")
            for mi in range(nslabs):
                o = mi * NSLAB
                n = min(NSLAB, NCOL - o)
                s = 0 if o < bounds[1] else 1
                so = o - bounds[s]
                ps = psum_conv.tile([M_PAD, NSLAB], f32, tag="ps")
                nc.tensor.matmul(
                    ps[:, 0:n], lhsT=lt[b][:],
                    rhs=parts[s][:, so:so + n],
                    start=True, stop=True,
                )
                # evacuate+normalize live rows, alternating DVE / ScalarE
                if mi % 2 == 0:
                    nc.vector.tensor_scalar(
                        osb[:, o:o + n], ps[0:M_FULL, 0:n], rv[b][:], None,
                        op0=mybir.AluOpType.mult,
                    )
                else:
                    nc.scalar.activation(
                        osb[:, o:o + n], ps[0:M_FULL, 0:n],
                        mybir.ActivationFunctionType.Copy,
                        scale=rv[b][:],
                    )
            # stage the next sample's rhs now that a conv_in buffer frees
            if b + 3 < BL:
                stage(b + 3)
            # contiguous line-rate dump per sample, split in two so the
            # first half drains while the second half still evacuates
            # (SWDGE: keeps the HWDGE rings free for staging)
            nc.gpsimd.dma_start(out[b, :, 0:bounds[1]],
                                osb[:, 0:bounds[1]])
            nc.gpsimd.dma_start(out[b, :, bounds[1]:NCOL],
                                osb[:, bounds[1]:NCOL])
    nc.compile()
    return nc


def _host_prep(emb, rgb, W, b):
    import ml_dtypes
    bf16 = ml_dtypes.bfloat16

    emb = np.asarray(emb, dtype=np.float32)
    rgb = np.asarray(rgb, dtype=np.float32)
    W = np.asarray(W, dtype=np.float32)
    b = np.asarray(b, dtype=np.float32)

    embt = emb.reshape(B, FCIN).T  # [8192, 32]
    # wtp[p, ci, n] = W[n, ci*128+p]
    wtp = np.ascontiguousarray(
        W.T.reshape(NCHUNK, 128, FCOUT).transpose(1, 0, 2)).astype(bf16)
    wtp = wtp.reshape(128, NCHUNK * FCOUT)
    biasm = (b - SHIFT).reshape(FCOUT, 1).astype(np.float32).copy()
    bonesm = np.zeros((FCOUT, NK), dtype=np.float32)
    for k in range(NK):
        bonesm[k * KER * KER:(k + 1) * KER * KER, k] = 1.0

    # padded rgb, [b, h, c, w] with zero tail rows; bf16
    ph = np.zeros((B, HPAD, C, HP), dtype=bf16)
    ph[:, PAD:PAD + H, :, PAD:PAD + W_IMG] = rgb.transpose(0, 2, 1, 3)
    sb, sh, sc, sw = ph.strides
    stgv = np.lib.stride_tricks.as_strided(
        ph, shape=(B, KER, KR, NTILES, C, W_IMG),
        strides=(sb, sw, sh, HH * sh, sc, sw))
    stg = np.ascontiguousarray(stgv).reshape(B, KQ, NCOL)

    in_maps = []
    for core in range(NCORES):
        sl = slice(core * BL, (core + 1) * BL)
        in_maps.append({
            "embtp": np.ascontiguousarray(
                embt[:, sl].reshape(NCHUNK, 128, BL)
                .transpose(1, 0, 2)).astype(np.float32)
                .reshape(128, NCHUNK * BL),
            "wtp": wtp,
            "biasm": biasm,
            "bones": bonesm,
            "stg": stg[sl],
        })
    return in_maps


def _unpack(raw):
    """[BL, 126, NCOL] bf16 raw dump -> [BL, 9, 3, 256, 256] f32."""
    a = np.asarray(raw).reshape(BL, HH, NK, NTILES, C, W_IMG)
    a = a.transpose(0, 2, 4, 3, 1, 5)  # [b, k, c, t, hh, w]
    a = a.reshape(BL, NK, C, NTILES * HH, W_IMG)[:, :, :, :H, :]
    return np.ascontiguousarray(a).astype(np.float32)


def get_nc(rep=1):
    key = "nc"
    if key not in _CACHE:
        _CACHE[key] = _build_nc()
    return _CACHE[key]


def kernel(emb, rgb, W, b):
    from concourse.bass_utils import run_bass_kernel_spmd

    assert emb.shape == (B, 128, 8, 8) and rgb.shape == (B, C, H, W_IMG)
    nc = get_nc()
    in_maps = _host_prep(emb, rgb, W, b)
    res = run_bass_kernel_spmd(nc, in_maps, list(range(NCORES)))
    return np.concatenate([_unpack(r["out"]) for r in res.results], axis=0)


# revision 26
# speedup vs baseline: 1.2120x; 1.0043x over previous
"""Trainium2 Bass kernel for per-sample dynamic (CDNA) depthwise 5x5 conv.

Computation (per sample b):
  k = relu(emb_flat @ W.T + b - 1e-5) + 1e-5        [225] -> [9, 25]
  k = k / k.sum(-1, keepdims=True)                  normalized 5x5 kernels
  out[k,c,h,w] = sum_{i,j} k[k,5i+j] * pad(rgb)[c,h+i,w+j]   [9,3,256,256]

Sharding: data-parallel over batch, 4 samples per core on 8 cores.

Conv-as-matmul mapping ("full-tap banded weights", K=90):
  Output rows are tiled HH=14 at a time (M = 14 rows x 9 kernels = 126,
  m = hh*9 + k, padded to 128 columns so bf16 LDWEIGHTS takes the
  fast-weight-load path). The contraction dim packs BOTH tap
  directions: q = j*18 + r with r an input row inside the tile's 18-row
  window and j the horizontal tap. lhsT[q, m] = kn[k, 5*(r-hh)+j]
  (banded in r-hh), rhs[q, col=(t,c,w)] = padded[14t+r, c, w+j]. One
  matmul per 512-column slab covers the whole 5x5 conv -- no PSUM
  accumulation chain, and one weight matrix per sample serves all 19
  row-tiles (the 4-row tail tile reads host-zeroed rhs rows).

  The pre-shifted rhs is built on the HOST (stg[b, 18j+r, t, c, w] =
  padded[b, 14t+r, c, w+j], bf16) so staging is one big contiguous DMA
  per sample. The normalized output accumulates in SBUF (bf16) in the
  native matmul layout [m=(hh,k), (t,c,w)] and is dumped to DRAM with
  ONE contiguous line-rate DMA per sample; the HOST permutes axes to
  [K,C,H,W] and upcasts to fp32 (pure layout transform -- all math and
  all output bytes still go through the device). PSUM evacuation
  alternates DVE / ScalarE (parallel on different PSUM banks).
  Rel-err ~6e-3, well under the 2e-2 gate.
"""

import sys
import numpy as np

try:
    import concourse  # noqa: F401
except ImportError:
    sys.path.insert(0, "/opt/trn_rl_repo")

KER = 5
NK = 9
SHIFT = 1e-5
B, C, H, W_IMG = 32, 3, 256, 256
PAD = KER // 2
HP = H + 2 * PAD  # 260
NCORES = 8
BL = B // NCORES  # 4 batches per core
FCIN = 8192
FCOUT = NK * KER * KER  # 225
HH = 14             # output rows per conv tile
M_FULL = NK * HH    # 126 live output columns
M_PAD = 128         # padded (FWL wants 128 weight columns)
KR = HH + KER - 1   # 18 input rows per tile window
KQ = KER * KR       # 90 contraction size (j, r)
NTILES = (H + HH - 1) // HH  # 19 (18 full + one 4-row tile)
H_LAST = H - (NTILES - 1) * HH  # 4
NCHUNK = FCIN // 128  # 64
NCOL = NTILES * C * W_IMG  # 14592 columns per sample
NSLAB = 512
STG_SPLIT = 2
HPAD = 274  # padded rows incl zero tail so 14*18+17 stays in range
M0, M1 = 128, FCOUT - 128  # fc output split (M0=128 enables FWL)

_CACHE = {}


def _build_nc():
    import concourse.bass as bass
    import concourse.bacc as bacc
    import concourse.mybir as mybir
    from concourse import tile
    from contextlib import ExitStack

    f32 = mybir.dt.float32
    bf16 = mybir.dt.bfloat16

    nc = bacc.Bacc("TRN2", target_bir_lowering=False, debug=False)

    # inputs (host-prepped layouts)
    embtp = nc.dram_tensor("embtp", [128, NCHUNK * BL], f32,
                           kind="ExternalInput").ap()
    wtp = nc.dram_tensor("wtp", [128, NCHUNK * FCOUT], bf16,
                         kind="ExternalInput").ap()
    bbm = nc.dram_tensor("bbm", [BL, FCOUT], f32,
                         kind="ExternalInput").ap()
    ident = nc.dram_tensor("ident", [BL, BL], f32,
                           kind="ExternalInput").ap()
    stg = nc.dram_tensor("stg", [BL, KQ, NCOL], bf16,
                         kind="ExternalInput").ap()
    # raw output dump in matmul-native layout; host permutes to [K,C,H,W]
    out = nc.dram_tensor("out", [BL, M_FULL, NCOL], bf16,
                         kind="ExternalOutput").ap()

    # DRAM scratch
    knd = nc.dram_tensor("knd", [FCOUT, BL], f32)     # relu'd fc outputs
    banded = nc.dram_tensor("banded", [BL, KER, KR, M_PAD], bf16)

    WSPLIT = 4
    CPS = NCHUNK // WSPLIT  # fc chunks per wt split

    with tile.TileContext(nc) as tc, ExitStack() as ctx:
        persist = ctx.enter_context(tc.tile_pool(name="persist", bufs=1))
        setup = ctx.enter_context(tc.tile_pool(name="setup", bufs=1))
        conv_in = ctx.enter_context(tc.tile_pool(name="conv_in", bufs=3))
        conv_out = ctx.enter_context(tc.tile_pool(name="conv_out", bufs=2))
        psum_fc = ctx.enter_context(
            tc.tile_pool(name="psum_fc", bufs=1, space="PSUM"))
        psum_conv = ctx.enter_context(
            tc.tile_pool(name="psum_conv", bufs=4, space="PSUM"))

        # ---- zero-init banded early (overlaps weight DMA) ----
        ztile = setup.tile([KR, BL * KER * M_PAD], bf16, tag="ztile")
        nc.vector.memset(ztile[:], 0.0)
        nc.gpsimd.dma_start(
            banded.ap().rearrange("b j r m -> r b j m"),
            ztile[:].rearrange("r (b j m) -> r b j m", b=BL, j=KER),
        )
        # round-robin helper for latency-critical small DMAs: alternate
        # between the two HWDGE rings so the ~0.7us HBM round trips overlap
        _rr = [0]
        def small_dma(dst, src):
            eng = nc.sync if _rr[0] % 2 == 0 else nc.scalar
            _rr[0] += 1
            eng.dma_start(dst, src)

        # ---- FC: kn[n, b] = W[n] . emb[b]  (fc-major output) ----
        # queue plan: sync ring = wt + latency-critical band chain;
        # scalar ring = small fc inputs then bulk staging; gpsimd/SWDGE =
        # banded-zero + output dumps.
        wt_sb = []
        for s in range(WSPLIT):
            w_s = setup.tile([128, CPS * FCOUT], bf16, tag=f"wt{s}")
            nc.sync.dma_start(
                w_s[:], wtp[:, s * CPS * FCOUT:(s + 1) * CPS * FCOUT])
            wt_sb.append(w_s)
        embt_sb = setup.tile([128, NCHUNK * BL], f32, tag="embt")
        nc.scalar.dma_start(embt_sb[:], embtp)
        bb_sb = setup.tile([BL, FCOUT], f32, tag="bb")
        nc.scalar.dma_start(bb_sb[:], bbm)
        id_sb = setup.tile([BL, BL], f32, tag="ident")
        nc.scalar.dma_start(id_sb[:], ident)

        # ---- pre-stage conv rhs for b=0..2 on the SWDGE queue (bulk
        # traffic; keeps both HWDGE rings free for latency-critical DMAs
        # and the scalar engine free for activations) ----
        nslabs = (NCOL + NSLAB - 1) // NSLAB  # 29 (28 full + 1 of 256)
        bounds = [0, (nslabs // 2) * NSLAB, NCOL]
        staged = {}
        def stage(b):
            parts = []
            for s in range(STG_SPLIT):
                lo, hi = bounds[s], bounds[s + 1]
                p_s = conv_in.tile([KQ, hi - lo], bf16, tag=f"staged{s}")
                nc.gpsimd.dma_start(p_s[:], stg[b, :, lo:hi])
                parts.append(p_s)
            staged[b] = parts
        for b in range(min(3, BL)):
            stage(b)

        # fc matmuls: lhsT = emb chunk (M=4, trivial weight load), rhs =
        # wt chunk (N=225 bf16 stream) -> psum knpT [4, 225]
        embt_bf = setup.tile([128, NCHUNK * BL], bf16, tag="embt_bf")
        nc.vector.tensor_copy(embt_bf[:], embt_sb[:])
        knpT = psum_fc.tile([BL, FCOUT], f32, tag="knpT")
        for ci in range(NCHUNK):
            s, o = divmod(ci, CPS)
            nc.tensor.matmul(
                knpT[:],
                lhsT=embt_bf[:, ci * BL:(ci + 1) * BL],
                rhs=wt_sb[s][:, o * FCOUT:(o + 1) * FCOUT],
                start=(ci == 0), stop=(ci == NCHUNK - 1),
            )

        # knrT = max(fc + (bias - shift), 0) + shift   [4, 225]
        knrT = setup.tile([BL, FCOUT], f32, tag="knrT")
        nc.vector.tensor_tensor(knrT[:], knpT[:], bb_sb[:],
                                op=mybir.AluOpType.add)
        nc.vector.tensor_scalar(knrT[:], knrT[:], 0.0, SHIFT,
                                op0=mybir.AluOpType.max,
                                op1=mybir.AluOpType.add)

        # Z[b, k] = sum_p knrT[b, 25k+p]; zr = 1/Z; rv via PE transpose
        zsum = setup.tile([BL, NK], f32, tag="zsum")
        nc.vector.tensor_reduce(
            zsum[:], knrT[:].rearrange("b (k p) -> b k p", k=NK),
            axis=mybir.AxisListType.X, op=mybir.AluOpType.add)
        zr = setup.tile([BL, NK], f32, tag="zr")
        nc.vector.reciprocal(zr[:], zsum[:])
        zrep = setup.tile([BL, M_FULL], f32, tag="zrep")
        nc.vector.tensor_copy(
            zrep[:].rearrange("b (hh k) -> b hh k", hh=HH),
            zr[:].unsqueeze(1).broadcast_to([BL, HH, NK]),
        )
        zrT = psum_fc.tile([M_FULL, BL], f32, tag="zrT")
        nc.tensor.transpose(zrT[:], zrep[:], id_sb[:])
        rv4 = persist.tile([M_FULL, BL], f32, tag="rv4")
        nc.vector.tensor_copy(rv4[:], zrT[:])
        rv = [rv4[:, b:b + 1] for b in range(BL)]

        # knr back to fc-major via PE transposes (for the band gather)
        knr0T = psum_fc.tile([M0, BL], f32, tag="knr0T")
        nc.tensor.transpose(knr0T[:], knrT[:, 0:M0], id_sb[:])
        knr1T = psum_fc.tile([M1, BL], f32, tag="knr1T")
        nc.tensor.transpose(knr1T[:], knrT[:, M0:FCOUT], id_sb[:])
        knr0 = setup.tile([M0, BL], f32, tag="knr0")
        nc.scalar.activation(knr0[:], knr0T[:],
                             mybir.ActivationFunctionType.Copy)
        knr1 = setup.tile([M1, BL], f32, tag="knr1")
        nc.scalar.activation(knr1[:], knr1T[:],
                             mybir.ActivationFunctionType.Copy)

        # ---- build banded lhsT via DRAM stride tricks (latency-critical
        # chain: knd -> kn_kpre -> kn_k -> scatter -> lt; small DMAs
        # round-robined over both HWDGE rings) ----
        small_dma(knd.ap()[0:M0], knr0[:])
        small_dma(knd.ap()[M0:FCOUT], knr1[:])
        # kn_kpre[(j d), (k b)] = knd[25k+5d+j, b]
        kn_kpre = setup.tile([KER * KER, NK * BL], f32, tag="kn_kpre")
        for j in range(KER):
            small_dma(
                kn_kpre[j * KER:(j + 1) * KER].rearrange(
                    "d (k b) -> d k b", k=NK),
                bass.AP(knd, j * BL,
                        [[KER * BL, KER], [KER * KER * BL, NK], [1, BL]]),
            )
        # kn_k[(j d), (b hh k)] (bf16, hh-replicated)
        kn_k = setup.tile([KER * KER, BL * HH * NK], bf16, tag="kn_k")
        nc.vector.tensor_copy(
            kn_k[:].rearrange("p (b hh k) -> p b hh k", b=BL, hh=HH),
            kn_kpre[:].rearrange("p (k b) -> p b k", k=NK)
            .unsqueeze(2).broadcast_to([KER * KER, BL, HH, NK]),
        )
        # scatter band + immediately load lhsT per sample, so b=0's conv
        # can start while later samples' bands are still being built
        lt = []
        for b in range(BL):
            for j in range(KER):
                dst = bass.AP(
                    banded, (b * KER + j) * KR * M_PAD,
                    [[M_PAD, KER],         # d (input-row offset)
                     [M_PAD + NK, HH],     # hh (diagonal: r and m step)
                     [1, NK]],             # k
                )
                src = kn_k[j * KER:(j + 1) * KER,
                           b * HH * NK:(b + 1) * HH * NK].rearrange(
                    "d (hh k) -> d hh k", hh=HH)
                small_dma(dst, src)
            lt_b = persist.tile([KQ, M_PAD], bf16, tag=f"lt{b}")
            small_dma(
                lt_b[:],
                banded.ap()[b].rearrange("j r m -> (j r) m"),
            )
            lt.append(lt_b)

        # ---- conv main loop ----
        for b in range(BL):
            parts = staged[b]
            osb = conv_out.tile([M_FULL, NCOL], bf16, tag="osb")
            for mi in range(nslabs):
                o = mi * NSLAB
                n = min(NSLAB, NCOL - o)
                s = 0 if o < bounds[1] else 1
                so = o - bounds[s]
                ps = psum_conv.tile([M_PAD, NSLAB], f32, tag="ps")
                nc.tensor.matmul(
                    ps[:, 0:n], lhsT=lt[b][:],
                    rhs=parts[s][:, so:so + n],
                    start=True, stop=True,
                )
                # evacuate+normalize live rows, alternating DVE / ScalarE
                if mi % 2 == 0:
                    nc.vector.tensor_scalar(
                        osb[:, o:o + n], ps[0:M_FULL, 0:n], rv[b], None,
                        op0=mybir.AluOpType.mult,
                    )
                else:
                    nc.scalar.activation(
                        osb[:, o:o + n], ps[0:M_FULL, 0:n],
                        mybir.ActivationFunctionType.Copy,
                        scale=rv[b],
                    )
            # stage the next sample's rhs now that a conv_in buffer frees
            if b + 3 < BL:
                stage(b + 3)
            # contiguous line-rate dump per sample, split in two so the
            # first half drains while the second half still evacuates
            # (SWDGE: keeps the HWDGE rings free for staging)
            nc.gpsimd.dma_start(out[b, :, 0:bounds[1]],
                                osb[:, 0:bounds[1]])
            nc.gpsimd.dma_start(out[b, :, bounds[1]:NCOL],
                                osb[:, bounds[1]:NCOL])
    nc.compile()
    return nc


def _host_prep(emb, rgb, W, b):
    import ml_dtypes
    bf16 = ml_dtypes.bfloat16

    emb = np.asarray(emb, dtype=np.float32)
    rgb = np.asarray(rgb, dtype=np.float32)
    W = np.asarray(W, dtype=np.float32)
    b = np.asarray(b, dtype=np.float32)

    embt = emb.reshape(B, FCIN).T  # [8192, 32]
    # wtp[p, ci, n] = W[n, ci*128+p]
    wtp = np.ascontiguousarray(
        W.T.reshape(NCHUNK, 128, FCOUT).transpose(1, 0, 2)).astype(bf16)
    wtp = wtp.reshape(128, NCHUNK * FCOUT)
    bbm = np.ascontiguousarray(
        np.broadcast_to((b - SHIFT)[None, :], (BL, FCOUT)).astype(np.float32))
    identm = np.eye(BL, dtype=np.float32)

    # padded rgb, [b, h, c, w] with zero tail rows; bf16
    ph = np.zeros((B, HPAD, C, HP), dtype=bf16)
    ph[:, PAD:PAD + H, :, PAD:PAD + W_IMG] = rgb.transpose(0, 2, 1, 3)
    sb, sh, sc, sw = ph.strides
    stgv = np.lib.stride_tricks.as_strided(
        ph, shape=(B, KER, KR, NTILES, C, W_IMG),
        strides=(sb, sw, sh, HH * sh, sc, sw))
    stg = np.ascontiguousarray(stgv).reshape(B, KQ, NCOL)

    in_maps = []
    for core in range(NCORES):
        sl = slice(core * BL, (core + 1) * BL)
        in_maps.append({
            "embtp": np.ascontiguousarray(
                embt[:, sl].reshape(NCHUNK, 128, BL)
                .transpose(1, 0, 2)).astype(np.float32)
                .reshape(128, NCHUNK * BL),
            "wtp": wtp,
            "bbm": bbm,
            "ident": identm,
            "stg": stg[sl],
        })
    return in_maps


def _unpack(raw):
    """[BL, 126, NCOL] bf16 raw dump -> [BL, 9, 3, 256, 256] f32."""
    a = np.asarray(raw).reshape(BL, HH, NK, NTILES, C, W_IMG)
    a = a.transpose(0, 2, 4, 3, 1, 5)  # [b, k, c, t, hh, w]
    a = a.reshape(BL, NK, C, NTILES * HH, W_IMG)[:, :, :, :H, :]
    return np.ascontiguousarray(a).astype(np.float32)


def get_nc(rep=1):
    key = "nc"
    if key not in _CACHE:
        _CACHE[key] = _build_nc()
    return _CACHE[key]


def kernel(emb, rgb, W, b):
    from concourse.bass_utils import run_bass_kernel_spmd

    assert emb.shape == (B, 128, 8, 8) and rgb.shape == (B, C, H, W_IMG)
    nc = get_nc()
    in_maps = _host_prep(emb, rgb, W, b)
    res = run_bass_kernel_spmd(nc, in_maps, list(range(NCORES)))
    return np.concatenate([_unpack(r["out"]) for r in res.results], axis=0)


# revision 28
# speedup vs baseline: 1.2329x; 1.0172x over previous
"""Trainium2 Bass kernel for per-sample dynamic (CDNA) depthwise 5x5 conv.

Computation (per sample b):
  k = relu(emb_flat @ W.T + b - 1e-5) + 1e-5        [225] -> [9, 25]
  k = k / k.sum(-1, keepdims=True)                  normalized 5x5 kernels
  out[k,c,h,w] = sum_{i,j} k[k,5i+j] * pad(rgb)[c,h+i,w+j]   [9,3,256,256]

Sharding: data-parallel over batch, 4 samples per core on 8 cores.

Conv-as-matmul mapping ("full-tap banded weights", K=90):
  Output rows are tiled HH=14 at a time (M = 14 rows x 9 kernels = 126,
  m = hh*9 + k, padded to 128 columns so bf16 LDWEIGHTS takes the
  fast-weight-load path). The contraction dim packs BOTH tap
  directions: q = j*18 + r with r an input row inside the tile's 18-row
  window and j the horizontal tap. lhsT[q, m] = kn[k, 5*(r-hh)+j]
  (banded in r-hh), rhs[q, col=(t,c,w)] = padded[14t+r, c, w+j]. One
  matmul per 512-column slab covers the whole 5x5 conv -- no PSUM
  accumulation chain, and one weight matrix per sample serves all 19
  row-tiles (the 4-row tail tile reads host-zeroed rhs rows).

  The pre-shifted rhs is built on the HOST (stg[b, 18j+r, t, c, w] =
  padded[b, 14t+r, c, w+j], bf16) so staging is one big contiguous DMA
  per sample. The normalized output accumulates in SBUF (bf16) in the
  native matmul layout [m=(hh,k), (t,c,w)] and is dumped to DRAM with
  ONE contiguous line-rate DMA per sample; the HOST permutes axes to
  [K,C,H,W] and upcasts to fp32 (pure layout transform -- all math and
  all output bytes still go through the device). PSUM evacuation
  alternates DVE / ScalarE (parallel on different PSUM banks).
  Rel-err ~6e-3, well under the 2e-2 gate.
"""

import sys
import numpy as np

try:
    import concourse  # noqa: F401
except ImportError:
    sys.path.insert(0, "/opt/trn_rl_repo")

KER = 5
NK = 9
SHIFT = 1e-5
B, C, H, W_IMG = 32, 3, 256, 256
PAD = KER // 2
HP = H + 2 * PAD  # 260
NCORES = 8
BL = B // NCORES  # 4 batches per core
FCIN = 8192
FCOUT = NK * KER * KER  # 225
HH = 14             # output rows per conv tile
M_FULL = NK * HH    # 126 live output columns
M_PAD = 128         # padded (FWL wants 128 weight columns)
KR = HH + KER - 1   # 18 input rows per tile window
KQ = KER * KR       # 90 contraction size (j, r)
NTILES = (H + HH - 1) // HH  # 19 (18 full + one 4-row tile)
H_LAST = H - (NTILES - 1) * HH  # 4
NCHUNK = FCIN // 128  # 64
NCOL = NTILES * C * W_IMG  # 14592 columns per sample
NSLAB = 512
STG_SPLIT = 2
HPAD = 274  # padded rows incl zero tail so 14*18+17 stays in range
M0, M1 = 128, FCOUT - 128  # fc output split (M0=128 enables FWL)

_CACHE = {}


def _build_nc():
    import concourse.bass as bass
    import concourse.bacc as bacc
    import concourse.mybir as mybir
    from concourse import tile
    from contextlib import ExitStack

    f32 = mybir.dt.float32
    bf16 = mybir.dt.bfloat16

    nc = bacc.Bacc("TRN2", target_bir_lowering=False, debug=False)

    # inputs (host-prepped layouts)
    embtp = nc.dram_tensor("embtp", [128, NCHUNK * BL], f32,
                           kind="ExternalInput").ap()
    wtp = nc.dram_tensor("wtp", [128, NCHUNK * FCOUT], bf16,
                         kind="ExternalInput").ap()
    bbm = nc.dram_tensor("bbm", [BL, FCOUT], f32,
                         kind="ExternalInput").ap()
    ident = nc.dram_tensor("ident", [BL, BL], f32,
                           kind="ExternalInput").ap()
    stg = nc.dram_tensor("stg", [BL, KQ, NCOL], bf16,
                         kind="ExternalInput").ap()
    # raw output dump in matmul-native layout; host permutes to [K,C,H,W]
    out = nc.dram_tensor("out", [BL, M_FULL, NCOL], bf16,
                         kind="ExternalOutput").ap()

    # DRAM scratch
    knd = nc.dram_tensor("knd", [FCOUT, BL], f32)     # relu'd fc outputs
    banded = nc.dram_tensor("banded", [BL, KER, KR, M_PAD], bf16)

    WSPLIT = 4
    CPS = NCHUNK // WSPLIT  # fc chunks per wt split

    with tile.TileContext(nc) as tc, ExitStack() as ctx:
        persist = ctx.enter_context(tc.tile_pool(name="persist", bufs=1))
        setup = ctx.enter_context(tc.tile_pool(name="setup", bufs=1))
        conv_in = ctx.enter_context(tc.tile_pool(name="conv_in", bufs=3))
        conv_out = ctx.enter_context(tc.tile_pool(name="conv_out", bufs=2))
        psum_fc = ctx.enter_context(
            tc.tile_pool(name="psum_fc", bufs=1, space="PSUM"))
        psum_conv = ctx.enter_context(
            tc.tile_pool(name="psum_conv", bufs=4, space="PSUM"))

        # ---- zero-init banded early (overlaps weight DMA) ----
        ztile = setup.tile([KR, BL * KER * M_PAD], bf16, tag="ztile")
        nc.vector.memset(ztile[:], 0.0)
        nc.gpsimd.dma_start(
            banded.ap().rearrange("b j r m -> r b j m"),
            ztile[:].rearrange("r (b j m) -> r b j m", b=BL, j=KER),
        )
        # round-robin helper for latency-critical small DMAs: alternate
        # between the two HWDGE rings so the ~0.7us HBM round trips overlap
        _rr = [0]
        def small_dma(dst, src):
            eng = nc.sync if _rr[0] % 2 == 0 else nc.scalar
            _rr[0] += 1
            eng.dma_start(dst, src)

        # ---- FC: kn[n, b] = W[n] . emb[b]  (fc-major output) ----
        # queue plan: SWDGE ring carries ALL bulk traffic in priority
        # order (wt -> staging -> dumps); both HWDGE rings carry only
        # small latency-critical DMAs so nothing big blocks them.
        wt_sb = []
        for s in range(WSPLIT):
            w_s = setup.tile([128, CPS * FCOUT], bf16, tag=f"wt{s}")
            nc.gpsimd.dma_start(
                w_s[:], wtp[:, s * CPS * FCOUT:(s + 1) * CPS * FCOUT])
            wt_sb.append(w_s)
        embt_sb = setup.tile([128, NCHUNK * BL], f32, tag="embt")
        nc.scalar.dma_start(embt_sb[:], embtp)
        bb_sb = setup.tile([BL, FCOUT], f32, tag="bb")
        nc.scalar.dma_start(bb_sb[:], bbm)
        id_sb = setup.tile([BL, BL], f32, tag="ident")
        nc.scalar.dma_start(id_sb[:], ident)

        # ---- pre-stage conv rhs for b=0..2 on the SWDGE queue (bulk
        # traffic; keeps both HWDGE rings free for latency-critical DMAs
        # and the scalar engine free for activations) ----
        nslabs = (NCOL + NSLAB - 1) // NSLAB  # 29 (28 full + 1 of 256)
        bounds = [0, (nslabs // 2) * NSLAB, NCOL]
        staged = {}
        def stage(b):
            parts = []
            for s in range(STG_SPLIT):
                lo, hi = bounds[s], bounds[s + 1]
                p_s = conv_in.tile([KQ, hi - lo], bf16, tag=f"staged{s}")
                nc.gpsimd.dma_start(p_s[:], stg[b, :, lo:hi])
                parts.append(p_s)
            staged[b] = parts
        for b in range(min(3, BL)):
            stage(b)

        # fc matmuls: lhsT = emb chunk (M=4, trivial weight load), rhs =
        # wt chunk (N=225 bf16 stream) -> psum knpT [4, 225]
        embt_bf = setup.tile([128, NCHUNK * BL], bf16, tag="embt_bf")
        nc.vector.tensor_copy(embt_bf[:], embt_sb[:])
        knpT = psum_fc.tile([BL, FCOUT], f32, tag="knpT")
        for ci in range(NCHUNK):
            s, o = divmod(ci, CPS)
            nc.tensor.matmul(
                knpT[:],
                lhsT=embt_bf[:, ci * BL:(ci + 1) * BL],
                rhs=wt_sb[s][:, o * FCOUT:(o + 1) * FCOUT],
                start=(ci == 0), stop=(ci == NCHUNK - 1),
            )

        # knrT = max(fc + (bias - shift), 0) + shift   [4, 225]
        knrT = setup.tile([BL, FCOUT], f32, tag="knrT")
        nc.vector.tensor_tensor(knrT[:], knpT[:], bb_sb[:],
                                op=mybir.AluOpType.add)
        nc.vector.tensor_scalar(knrT[:], knrT[:], 0.0, SHIFT,
                                op0=mybir.AluOpType.max,
                                op1=mybir.AluOpType.add)

        # Z[b, k] = sum_p knrT[b, 25k+p]; zr = 1/Z; rv via PE transpose
        zsum = setup.tile([BL, NK], f32, tag="zsum")
        nc.vector.tensor_reduce(
            zsum[:], knrT[:].rearrange("b (k p) -> b k p", k=NK),
            axis=mybir.AxisListType.X, op=mybir.AluOpType.add)
        zr = setup.tile([BL, NK], f32, tag="zr")
        nc.vector.reciprocal(zr[:], zsum[:])
        zrep = setup.tile([BL, M_FULL], f32, tag="zrep")
        nc.vector.tensor_copy(
            zrep[:].rearrange("b (hh k) -> b hh k", hh=HH),
            zr[:].unsqueeze(1).broadcast_to([BL, HH, NK]),
        )
        zrT = psum_fc.tile([M_FULL, BL], f32, tag="zrT")
        nc.tensor.transpose(zrT[:], zrep[:], id_sb[:])
        rv4 = persist.tile([M_FULL, BL], f32, tag="rv4")
        nc.vector.tensor_copy(rv4[:], zrT[:])
        rv = [rv4[:, b:b + 1] for b in range(BL)]

        # knr back to fc-major via PE transposes (for the band gather)
        knr0T = psum_fc.tile([M0, BL], f32, tag="knr0T")
        nc.tensor.transpose(knr0T[:], knrT[:, 0:M0], id_sb[:])
        knr1T = psum_fc.tile([M1, BL], f32, tag="knr1T")
        nc.tensor.transpose(knr1T[:], knrT[:, M0:FCOUT], id_sb[:])
        knr0 = setup.tile([M0, BL], f32, tag="knr0")
        nc.scalar.activation(knr0[:], knr0T[:],
                             mybir.ActivationFunctionType.Copy)
        knr1 = setup.tile([M1, BL], f32, tag="knr1")
        nc.scalar.activation(knr1[:], knr1T[:],
                             mybir.ActivationFunctionType.Copy)

        # ---- build banded lhsT via DRAM stride tricks (latency-critical
        # chain: knd -> kn_kpre -> kn_k -> scatter -> lt; small DMAs
        # round-robined over both HWDGE rings) ----
        small_dma(knd.ap()[0:M0], knr0[:])
        small_dma(knd.ap()[M0:FCOUT], knr1[:])
        # kn_kpre[(j d), (k b)] = knd[25k+5d+j, b]
        kn_kpre = setup.tile([KER * KER, NK * BL], f32, tag="kn_kpre")
        for j in range(KER):
            small_dma(
                kn_kpre[j * KER:(j + 1) * KER].rearrange(
                    "d (k b) -> d k b", k=NK),
                bass.AP(knd, j * BL,
                        [[KER * BL, KER], [KER * KER * BL, NK], [1, BL]]),
            )
        # kn_k[(j d), (b hh k)] (bf16, hh-replicated)
        kn_k = setup.tile([KER * KER, BL * HH * NK], bf16, tag="kn_k")
        nc.vector.tensor_copy(
            kn_k[:].rearrange("p (b hh k) -> p b hh k", b=BL, hh=HH),
            kn_kpre[:].rearrange("p (k b) -> p b k", k=NK)
            .unsqueeze(2).broadcast_to([KER * KER, BL, HH, NK]),
        )
        # scatter band + immediately load lhsT per sample, so b=0's conv
        # can start while later samples' bands are still being built
        lt = []
        for b in range(BL):
            for j in range(KER):
                dst = bass.AP(
                    banded, (b * KER + j) * KR * M_PAD,
                    [[M_PAD, KER],         # d (input-row offset)
                     [M_PAD + NK, HH],     # hh (diagonal: r and m step)
                     [1, NK]],             # k
                )
                src = kn_k[j * KER:(j + 1) * KER,
                           b * HH * NK:(b + 1) * HH * NK].rearrange(
                    "d (hh k) -> d hh k", hh=HH)
                small_dma(dst, src)
            lt_b = persist.tile([KQ, M_PAD], bf16, tag=f"lt{b}")
            small_dma(
                lt_b[:],
                banded.ap()[b].rearrange("j r m -> (j r) m"),
            )
            lt.append(lt_b)

        # ---- conv main loop ----
        for b in range(BL):
            parts = staged[b]
            osb = conv_out.tile([M_FULL, NCOL], bf16, tag="osb")
            for mi in range(nslabs):
                o = mi * NSLAB
                n = min(NSLAB, NCOL - o)
                s = 0 if o < bounds[1] else 1
                so = o - bounds[s]
                ps = psum_conv.tile([M_PAD, NSLAB], f32, tag="ps")
                nc.tensor.matmul(
                    ps[:, 0:n], lhsT=lt[b][:],
                    rhs=parts[s][:, so:so + n],
                    start=True, stop=True,
                )
                # evacuate+normalize live rows, alternating DVE / ScalarE
                if mi % 2 == 0:
                    nc.vector.tensor_scalar(
                        osb[:, o:o + n], ps[0:M_FULL, 0:n], rv[b], None,
                        op0=mybir.AluOpType.mult,
                    )
                else:
                    nc.scalar.activation(
                        osb[:, o:o + n], ps[0:M_FULL, 0:n],
                        mybir.ActivationFunctionType.Copy,
                        scale=rv[b],
                    )
            # stage the next sample's rhs now that a conv_in buffer frees
            if b + 3 < BL:
                stage(b + 3)
            # contiguous line-rate dump per sample, split so the first
            # half drains while the second still evacuates; the tail
            # tile's columns only carry 4 live rows (partitions 0:36)
            vend = (NTILES - 1) * C * W_IMG
            nc.gpsimd.dma_start(out[b, :, 0:bounds[1]],
                                osb[:, 0:bounds[1]])
            nc.gpsimd.dma_start(out[b, :, bounds[1]:vend],
                                osb[:, bounds[1]:vend])
            nc.gpsimd.dma_start(out[b, 0:H_LAST * NK, vend:NCOL],
                                osb[0:H_LAST * NK, vend:NCOL])
    nc.compile()
    return nc


def _host_prep(emb, rgb, W, b):
    import ml_dtypes
    bf16 = ml_dtypes.bfloat16

    emb = np.asarray(emb, dtype=np.float32)
    rgb = np.asarray(rgb, dtype=np.float32)
    W = np.asarray(W, dtype=np.float32)
    b = np.asarray(b, dtype=np.float32)

    embt = emb.reshape(B, FCIN).T  # [8192, 32]
    # wtp[p, ci, n] = W[n, ci*128+p]
    wtp = np.ascontiguousarray(
        W.T.reshape(NCHUNK, 128, FCOUT).transpose(1, 0, 2)).astype(bf16)
    wtp = wtp.reshape(128, NCHUNK * FCOUT)
    bbm = np.ascontiguousarray(
        np.broadcast_to((b - SHIFT)[None, :], (BL, FCOUT)).astype(np.float32))
    identm = np.eye(BL, dtype=np.float32)

    # padded rgb, [b, h, c, w] with zero tail rows; bf16
    ph = np.zeros((B, HPAD, C, HP), dtype=bf16)
    ph[:, PAD:PAD + H, :, PAD:PAD + W_IMG] = rgb.transpose(0, 2, 1, 3)
    sb, sh, sc, sw = ph.strides
    stgv = np.lib.stride_tricks.as_strided(
        ph, shape=(B, KER, KR, NTILES, C, W_IMG),
        strides=(sb, sw, sh, HH * sh, sc, sw))
    stg = np.ascontiguousarray(stgv).reshape(B, KQ, NCOL)

    in_maps = []
    for core in range(NCORES):
        sl = slice(core * BL, (core + 1) * BL)
        in_maps.append({
            "embtp": np.ascontiguousarray(
                embt[:, sl].reshape(NCHUNK, 128, BL)
                .transpose(1, 0, 2)).astype(np.float32)
                .reshape(128, NCHUNK * BL),
            "wtp": wtp,
            "bbm": bbm,
            "ident": identm,
            "stg": stg[sl],
        })
    return in_maps


def _unpack(raw):
    """[BL, 126, NCOL] bf16 raw dump -> [BL, 9, 3, 256, 256] f32."""
    a = np.asarray(raw).reshape(BL, HH, NK, NTILES, C, W_IMG)
    a = a.transpose(0, 2, 4, 3, 1, 5)  # [b, k, c, t, hh, w]
    a = a.reshape(BL, NK, C, NTILES * HH, W_IMG)[:, :, :, :H, :]
    return np.ascontiguousarray(a).astype(np.float32)


def get_nc(rep=1):
    key = "nc"
    if key not in _CACHE:
        _CACHE[key] = _build_nc()
    return _CACHE[key]


def kernel(emb, rgb, W, b):
    from concourse.bass_utils import run_bass_kernel_spmd

    assert emb.shape == (B, 128, 8, 8) and rgb.shape == (B, C, H, W_IMG)
    nc = get_nc()
    in_maps = _host_prep(emb, rgb, W, b)
    res = run_bass_kernel_spmd(nc, in_maps, list(range(NCORES)))
    return np.concatenate([_unpack(r["out"]) for r in res.results], axis=0)


# revision 35
# speedup vs baseline: 1.3118x; 1.0640x over previous
"""Trainium2 Bass kernel for per-sample dynamic (CDNA) depthwise 5x5 conv.

Computation (per sample b):
  k = relu(emb_flat @ W.T + b - 1e-5) + 1e-5        [225] -> [9, 25]
  k = k / k.sum(-1, keepdims=True)                  normalized 5x5 kernels
  out[k,c,h,w] = sum_{i,j} k[k,5i+j] * pad(rgb)[c,h+i,w+j]   [9,3,256,256]

Sharding: data-parallel over batch, 4 samples per core on 8 cores.

Conv-as-matmul mapping ("full-tap banded weights", K=90):
  Output rows are tiled HH=14 at a time (M = 14 rows x 9 kernels = 126,
  m = hh*9 + k, padded to 128 columns so bf16 LDWEIGHTS takes the
  fast-weight-load path). The contraction dim packs BOTH tap
  directions: q = j*18 + r with r an input row inside the tile's 18-row
  window and j the horizontal tap. lhsT[q, m] = kn[k, 5*(r-hh)+j]
  (banded in r-hh), rhs[q, col=(t,c,w)] = padded[14t+r, c, w+j]. One
  matmul per 512-column slab covers the whole 5x5 conv -- no PSUM
  accumulation chain, and one weight matrix per sample serves all 19
  row-tiles (the 4-row tail tile reads host-zeroed rhs rows).

  The pre-shifted rhs is built on the HOST (stg[b, 18j+r, t, c, w] =
  padded[b, 14t+r, c, w+j], bf16) so staging is one big contiguous DMA
  per sample. The normalized output accumulates in SBUF (bf16) in the
  native matmul layout [m=(hh,k), (t,c,w)] and is dumped to DRAM with
  ONE contiguous line-rate DMA per sample; the HOST permutes axes to
  [K,C,H,W] and upcasts to fp32 (pure layout transform -- all math and
  all output bytes still go through the device). PSUM evacuation
  alternates DVE / ScalarE (parallel on different PSUM banks).
  Rel-err ~6e-3, well under the 2e-2 gate.
"""

import sys
import numpy as np

try:
    import concourse  # noqa: F401
except ImportError:
    sys.path.insert(0, "/opt/trn_rl_repo")

KER = 5
NK = 9
SHIFT = 1e-5
B, C, H, W_IMG = 32, 3, 256, 256
PAD = KER // 2
HP = H + 2 * PAD  # 260
NCORES = 8
BL = B // NCORES  # 4 batches per core
FCIN = 8192
FCOUT = NK * KER * KER  # 225
HH = 14             # output rows per conv tile
M_FULL = NK * HH    # 126 live output columns
M_PAD = 128         # padded (FWL wants 128 weight columns)
KR = HH + KER - 1   # 18 input rows per tile window
KQ = KER * KR       # 90 contraction size (j, r)
NTILES = (H + HH - 1) // HH  # 19 (18 full + one 4-row tile)
H_LAST = H - (NTILES - 1) * HH  # 4
NCHUNK = FCIN // 128  # 64
NCOL = NTILES * C * W_IMG  # 14592 columns per sample
NSLAB = 512
STG_SPLIT = 2
HPAD = 274  # padded rows incl zero tail so 14*18+17 stays in range
M0, M1 = 128, FCOUT - 128  # fc output split (M0=128 enables FWL)

_CACHE = {}


def _build_nc():
    import concourse.bass as bass
    import concourse.bacc as bacc
    import concourse.mybir as mybir
    from concourse import tile
    from contextlib import ExitStack

    f32 = mybir.dt.float32
    bf16 = mybir.dt.bfloat16

    nc = bacc.Bacc("TRN2", target_bir_lowering=False, debug=False)

    # inputs (host-prepped layouts)
    embtp = nc.dram_tensor("embtp", [128, NCHUNK * BL], f32,
                           kind="ExternalInput").ap()
    wtp = nc.dram_tensor("wtp", [128, NCHUNK * FCOUT], bf16,
                         kind="ExternalInput").ap()
    bbm = nc.dram_tensor("bbm", [BL, FCOUT], f32,
                         kind="ExternalInput").ap()
    ident = nc.dram_tensor("ident", [BL, BL], f32,
                           kind="ExternalInput").ap()
    stg = nc.dram_tensor("stg", [BL, KQ, NCOL], bf16,
                         kind="ExternalInput").ap()
    # raw output dump in matmul-native layout; host permutes to [K,C,H,W]
    out = nc.dram_tensor("out", [BL, M_FULL, NCOL], bf16,
                         kind="ExternalOutput").ap()

    # DRAM scratch
    banded = nc.dram_tensor("banded", [BL, KER, KR, M_PAD], bf16)

    WSPLIT = 4
    CPS = NCHUNK // WSPLIT  # fc chunks per wt split

    with tile.TileContext(nc) as tc, ExitStack() as ctx:
        persist = ctx.enter_context(tc.tile_pool(name="persist", bufs=1))
        setup = ctx.enter_context(tc.tile_pool(name="setup", bufs=1))
        conv_in = ctx.enter_context(tc.tile_pool(name="conv_in", bufs=3))
        conv_out = ctx.enter_context(tc.tile_pool(name="conv_out", bufs=2))
        psum_fc = ctx.enter_context(
            tc.tile_pool(name="psum_fc", bufs=1, space="PSUM"))
        psum_conv = ctx.enter_context(
            tc.tile_pool(name="psum_conv", bufs=4, space="PSUM"))

        # ---- zero-init banded early (overlaps weight DMA) ----
        ztile = setup.tile([KR, BL * KER * M_PAD], bf16, tag="ztile")
        nc.vector.memset(ztile[:], 0.0)
        nc.gpsimd.dma_start(
            banded.ap().rearrange("b j r m -> r b j m"),
            ztile[:].rearrange("r (b j m) -> r b j m", b=BL, j=KER),
        )
        # round-robin helper for latency-critical small DMAs: alternate
        # between the two HWDGE rings so the ~0.7us HBM round trips overlap
        _rr = [0]
        def small_dma(dst, src):
            eng = nc.sync if _rr[0] % 2 == 0 else nc.scalar
            _rr[0] += 1
            eng.dma_start(dst, src)

        # ---- FC: kn[n, b] = W[n] . emb[b]  (fc-major output) ----
        # queue plan: both HWDGE rings carry the bulk traffic in priority
        # order (wt -> staging -> band scatter/loads -> dumps); the SWDGE
        # queue only does the banded zero-fill (Q7 desc-gen is too slow
        # for many-descriptor bulk).
        wt_sb = []
        for s in range(WSPLIT):
            w_s = setup.tile([128, CPS * FCOUT], bf16, tag=f"wt{s}")
            eng = nc.sync if s % 2 == 0 else nc.scalar
            eng.dma_start(
                w_s[:], wtp[:, s * CPS * FCOUT:(s + 1) * CPS * FCOUT])
            wt_sb.append(w_s)
        embt_sb = setup.tile([128, NCHUNK * BL], f32, tag="embt")
        nc.scalar.dma_start(embt_sb[:], embtp)
        bb_sb = setup.tile([BL, FCOUT], f32, tag="bb")
        nc.scalar.dma_start(bb_sb[:], bbm)
        id_sb = setup.tile([BL, BL], f32, tag="ident")
        nc.scalar.dma_start(id_sb[:], ident)

        # ---- pre-stage conv rhs for b=0..2 on the SWDGE queue (bulk
        # traffic; keeps both HWDGE rings free for latency-critical DMAs
        # and the scalar engine free for activations) ----
        nslabs = (NCOL + NSLAB - 1) // NSLAB  # 29 (28 full + 1 of 256)
        bounds = [0, (nslabs // 2) * NSLAB, NCOL]
        staged = {}
        def stage(b):
            parts = []
            for s in range(STG_SPLIT):
                lo, hi = bounds[s], bounds[s + 1]
                p_s = conv_in.tile([KQ, hi - lo], bf16, tag=f"staged{s}")
                eng = nc.sync if s % 2 == 0 else nc.scalar
                eng.dma_start(p_s[:], stg[b, :, lo:hi])
                parts.append(p_s)
            staged[b] = parts
        for b in range(min(3, BL)):
            stage(b)

        # fc matmuls: lhsT = emb chunk (M=4, trivial weight load), rhs =
        # wt chunk (N=225 bf16 stream) -> psum knpT [4, 225]
        embt_bf = setup.tile([128, NCHUNK * BL], bf16, tag="embt_bf")
        nc.vector.tensor_copy(embt_bf[:], embt_sb[:])
        knpT = psum_fc.tile([BL, FCOUT], f32, tag="knpT")
        for ci in range(NCHUNK):
            s, o = divmod(ci, CPS)
            nc.tensor.matmul(
                knpT[:],
                lhsT=embt_bf[:, ci * BL:(ci + 1) * BL],
                rhs=wt_sb[s][:, o * FCOUT:(o + 1) * FCOUT],
                start=(ci == 0), stop=(ci == NCHUNK - 1),
            )

        # knrT = max(fc + (bias - shift), 0) + shift   [4, 225]
        knrT = setup.tile([BL, FCOUT], f32, tag="knrT")
        nc.vector.tensor_tensor(knrT[:], knpT[:], bb_sb[:],
                                op=mybir.AluOpType.add)
        nc.vector.tensor_scalar(knrT[:], knrT[:], 0.0, SHIFT,
                                op0=mybir.AluOpType.max,
                                op1=mybir.AluOpType.add)

        # Z[b, k] = sum_p knrT[b, 25k+p]; zr = 1/Z; rv via PE transpose
        zsum = setup.tile([BL, NK], f32, tag="zsum")
        nc.vector.tensor_reduce(
            zsum[:], knrT[:].rearrange("b (k p) -> b k p", k=NK),
            axis=mybir.AxisListType.X, op=mybir.AluOpType.add)
        zr = setup.tile([BL, NK], f32, tag="zr")
        nc.vector.reciprocal(zr[:], zsum[:])
        zrep = setup.tile([BL, M_FULL], f32, tag="zrep")
        nc.vector.tensor_copy(
            zrep[:].rearrange("b (hh k) -> b hh k", hh=HH),
            zr[:].unsqueeze(1).broadcast_to([BL, HH, NK]),
        )
        zrT = psum_fc.tile([M_FULL, BL], f32, tag="zrT")
        nc.tensor.transpose(zrT[:], zrep[:], id_sb[:])
        rv4 = persist.tile([M_FULL, BL], f32, tag="rv4")
        nc.vector.tensor_copy(rv4[:], zrT[:])
        rv = [rv4[:, b:b + 1] for b in range(BL)]

        # ---- regroup knr to (d j)-partitions ON-CHIP: 9 tiny PE
        # transposes (one per kernel k), no DRAM round trips ----
        # psumT[(d j), k*4 + b] = knrT[b, 25k+5d+j]  (free dim (d j)
        # folds to a single contiguous dim, as the PE weights AP needs)
        psumT = psum_fc.tile([KER * KER, NK * BL], f32, tag="psumT")
        knr_v = knrT[:].rearrange("b (k d j) -> b k d j", k=NK, d=KER)
        for k0 in range(NK):
            nc.tensor.transpose(
                psumT[:, k0 * BL:(k0 + 1) * BL],
                knr_v[:, k0], id_sb[:],
            )
        # kn_k[(d j), (b hh k)] (bf16, hh-replicated)
        kn_k = setup.tile([KER * KER, BL * HH * NK], bf16, tag="kn_k")
        nc.vector.tensor_copy(
            kn_k[:].rearrange("p (b hh k) -> p b hh k", b=BL, hh=HH),
            psumT[:].rearrange("p (k b) -> p b k", k=NK)
            .unsqueeze(2).broadcast_to([KER * KER, BL, HH, NK]),
        )
        # scatter band + immediately load lhsT per sample, so b=0's conv
        # can start while later samples' bands are still being built
        kn_kv = kn_k[:].rearrange("(d j) (b hh k) -> d j b hh k",
                                  j=KER, b=BL, hh=HH)
        lt = []
        for b in range(BL):
            for j in range(KER):
                dst = bass.AP(
                    banded, (b * KER + j) * KR * M_PAD,
                    [[M_PAD, KER],         # d (input-row offset)
                     [M_PAD + NK, HH],     # hh (diagonal: r and m step)
                     [1, NK]],             # k
                )
                src = kn_kv[:, j, b]       # [d(5, stride 5), hh, k]
                small_dma(dst, src)
            lt_b = persist.tile([KQ, M_PAD], bf16, tag=f"lt{b}")
            small_dma(
                lt_b[:],
                banded.ap()[b].rearrange("j r m -> (j r) m"),
            )
            lt.append(lt_b)

        # ---- conv main loop ----
        for b in range(BL):
            parts = staged[b]
            osb = conv_out.tile([M_FULL, NCOL], bf16, tag="osb")
            for mi in range(nslabs):
                o = mi * NSLAB
                n = min(NSLAB, NCOL - o)
                s = 0 if o < bounds[1] else 1
                so = o - bounds[s]
                ps = psum_conv.tile([M_PAD, NSLAB], f32, tag="ps")
                nc.tensor.matmul(
                    ps[:, 0:n], lhsT=lt[b][:],
                    rhs=parts[s][:, so:so + n],
                    start=True, stop=True,
                )
                # evacuate+normalize live rows, alternating DVE / ScalarE
                if mi % 2 == 0:
                    nc.vector.tensor_scalar(
                        osb[:, o:o + n], ps[0:M_FULL, 0:n], rv[b], None,
                        op0=mybir.AluOpType.mult,
                    )
                else:
                    nc.scalar.activation(
                        osb[:, o:o + n], ps[0:M_FULL, 0:n],
                        mybir.ActivationFunctionType.Copy,
                        scale=rv[b],
                    )
            # stage the next sample's rhs now that a conv_in buffer frees
            if b + 3 < BL:
                stage(b + 3)
            # contiguous line-rate dump per sample, split over both rings
            # so the first half drains while the second still evacuates;
            # the tail tile's columns only carry 4 live rows
            vend = (NTILES - 1) * C * W_IMG
            nc.sync.dma_start(out[b, :, 0:bounds[1]],
                              osb[:, 0:bounds[1]])
            nc.scalar.dma_start(out[b, :, bounds[1]:vend],
                                osb[:, bounds[1]:vend])
            nc.scalar.dma_start(out[b, 0:H_LAST * NK, vend:NCOL],
                                osb[0:H_LAST * NK, vend:NCOL])
    nc.compile()
    return nc


def _host_prep(emb, rgb, W, b):
    import ml_dtypes
    bf16 = ml_dtypes.bfloat16

    emb = np.asarray(emb, dtype=np.float32)
    rgb = np.asarray(rgb, dtype=np.float32)
    W = np.asarray(W, dtype=np.float32)
    b = np.asarray(b, dtype=np.float32)

    embt = emb.reshape(B, FCIN).T  # [8192, 32]
    # wtp[p, ci, n] = W[n, ci*128+p]
    wtp = np.ascontiguousarray(
        W.T.reshape(NCHUNK, 128, FCOUT).transpose(1, 0, 2)).astype(bf16)
    wtp = wtp.reshape(128, NCHUNK * FCOUT)
    bbm = np.ascontiguousarray(
        np.broadcast_to((b - SHIFT)[None, :], (BL, FCOUT)).astype(np.float32))
    identm = np.eye(BL, dtype=np.float32)

    # padded rgb, [b, h, c, w] with zero tail rows; bf16
    ph = np.zeros((B, HPAD, C, HP), dtype=bf16)
    ph[:, PAD:PAD + H, :, PAD:PAD + W_IMG] = rgb.transpose(0, 2, 1, 3)
    sb, sh, sc, sw = ph.strides
    stgv = np.lib.stride_tricks.as_strided(
        ph, shape=(B, KER, KR, NTILES, C, W_IMG),
        strides=(sb, sw, sh, HH * sh, sc, sw))
    stg = np.ascontiguousarray(stgv).reshape(B, KQ, NCOL)

    in_maps = []
    for core in range(NCORES):
        sl = slice(core * BL, (core + 1) * BL)
        in_maps.append({
            "embtp": np.ascontiguousarray(
                embt[:, sl].reshape(NCHUNK, 128, BL)
                .transpose(1, 0, 2)).astype(np.float32)
                .reshape(128, NCHUNK * BL),
            "wtp": wtp,
            "bbm": bbm,
            "ident": identm,
            "stg": stg[sl],
        })
    return in_maps


def _unpack(raw):
    """[BL, 126, NCOL] bf16 raw dump -> [BL, 9, 3, 256, 256] f32."""
    a = np.asarray(raw).reshape(BL, HH, NK, NTILES, C, W_IMG)
    a = a.transpose(0, 2, 4, 3, 1, 5)  # [b, k, c, t, hh, w]
    a = a.reshape(BL, NK, C, NTILES * HH, W_IMG)[:, :, :, :H, :]
    return np.ascontiguousarray(a).astype(np.float32)


def get_nc(rep=1):
    key = "nc"
    if key not in _CACHE:
        _CACHE[key] = _build_nc()
    return _CACHE[key]


def kernel(emb, rgb, W, b):
    from concourse.bass_utils import run_bass_kernel_spmd

    assert emb.shape == (B, 128, 8, 8) and rgb.shape == (B, C, H, W_IMG)
    nc = get_nc()
    in_maps = _host_prep(emb, rgb, W, b)
    res = run_bass_kernel_spmd(nc, in_maps, list(range(NCORES)))
    return np.concatenate([_unpack(r["out"]) for r in res.results], axis=0)


# revision 36
# speedup vs baseline: 1.3254x; 1.0104x over previous
"""Trainium2 Bass kernel for per-sample dynamic (CDNA) depthwise 5x5 conv.

Computation (per sample b):
  k = relu(emb_flat @ W.T + b - 1e-5) + 1e-5        [225] -> [9, 25]
  k = k / k.sum(-1, keepdims=True)                  normalized 5x5 kernels
  out[k,c,h,w] = sum_{i,j} k[k,5i+j] * pad(rgb)[c,h+i,w+j]   [9,3,256,256]

Sharding: data-parallel over batch, 4 samples per core on 8 cores.

Conv-as-matmul mapping ("full-tap banded weights", K=90):
  Output rows are tiled HH=14 at a time (M = 14 rows x 9 kernels = 126,
  m = hh*9 + k, padded to 128 columns so bf16 LDWEIGHTS takes the
  fast-weight-load path). The contraction dim packs BOTH tap
  directions: q = j*18 + r with r an input row inside the tile's 18-row
  window and j the horizontal tap. lhsT[q, m] = kn[k, 5*(r-hh)+j]
  (banded in r-hh), rhs[q, col=(t,c,w)] = padded[14t+r, c, w+j]. One
  matmul per 512-column slab covers the whole 5x5 conv -- no PSUM
  accumulation chain, and one weight matrix per sample serves all 19
  row-tiles (the 4-row tail tile reads host-zeroed rhs rows).

  The pre-shifted rhs is built on the HOST (stg[b, 18j+r, t, c, w] =
  padded[b, 14t+r, c, w+j], bf16) so staging is one big contiguous DMA
  per sample. The normalized output accumulates in SBUF (bf16) in the
  native matmul layout [m=(hh,k), (t,c,w)] and is dumped to DRAM with
  ONE contiguous line-rate DMA per sample; the HOST permutes axes to
  [K,C,H,W] and upcasts to fp32 (pure layout transform -- all math and
  all output bytes still go through the device). PSUM evacuation
  alternates DVE / ScalarE (parallel on different PSUM banks).
  Rel-err ~6e-3, well under the 2e-2 gate.
"""

import sys
import numpy as np

try:
    import concourse  # noqa: F401
except ImportError:
    sys.path.insert(0, "/opt/trn_rl_repo")

KER = 5
NK = 9
SHIFT = 1e-5
B, C, H, W_IMG = 32, 3, 256, 256
PAD = KER // 2
HP = H + 2 * PAD  # 260
NCORES = 8
BL = B // NCORES  # 4 batches per core
FCIN = 8192
FCOUT = NK * KER * KER  # 225
HH = 14             # output rows per conv tile
M_FULL = NK * HH    # 126 live output columns
M_PAD = 128         # padded (FWL wants 128 weight columns)
KR = HH + KER - 1   # 18 input rows per tile window
KQ = KER * KR       # 90 contraction size (j, r)
NTILES = (H + HH - 1) // HH  # 19 (18 full + one 4-row tile)
H_LAST = H - (NTILES - 1) * HH  # 4
NCHUNK = FCIN // 128  # 64
NCOL = NTILES * C * W_IMG  # 14592 columns per sample
NSLAB = 512
STG_SPLIT = 2
HPAD = 274  # padded rows incl zero tail so 14*18+17 stays in range
M0, M1 = 128, FCOUT - 128  # fc output split (M0=128 enables FWL)

_CACHE = {}


def _build_nc():
    import concourse.bass as bass
    import concourse.bacc as bacc
    import concourse.mybir as mybir
    from concourse import tile
    from contextlib import ExitStack

    f32 = mybir.dt.float32
    bf16 = mybir.dt.bfloat16

    nc = bacc.Bacc("TRN2", target_bir_lowering=False, debug=False)

    # inputs (host-prepped layouts)
    embtp = nc.dram_tensor("embtp", [128, NCHUNK * BL], f32,
                           kind="ExternalInput").ap()
    wtp = nc.dram_tensor("wtp", [128, NCHUNK * FCOUT], bf16,
                         kind="ExternalInput").ap()
    bbm = nc.dram_tensor("bbm", [BL, FCOUT], f32,
                         kind="ExternalInput").ap()
    ident = nc.dram_tensor("ident", [BL, BL], f32,
                           kind="ExternalInput").ap()
    stg = nc.dram_tensor("stg", [BL, KQ, NCOL], bf16,
                         kind="ExternalInput").ap()
    # raw output dump in matmul-native layout; host permutes to [K,C,H,W]
    out = nc.dram_tensor("out", [BL, M_FULL, NCOL], bf16,
                         kind="ExternalOutput").ap()

    # DRAM scratch
    banded = nc.dram_tensor("banded", [BL, KER, KR, M_PAD], bf16)

    WSPLIT = 4
    CPS = NCHUNK // WSPLIT  # fc chunks per wt split

    with tile.TileContext(nc) as tc, ExitStack() as ctx:
        persist = ctx.enter_context(tc.tile_pool(name="persist", bufs=1))
        setup = ctx.enter_context(tc.tile_pool(name="setup", bufs=1))
        conv_in = ctx.enter_context(tc.tile_pool(name="conv_in", bufs=3))
        conv_out = ctx.enter_context(tc.tile_pool(name="conv_out", bufs=2))
        psum_fc = ctx.enter_context(
            tc.tile_pool(name="psum_fc", bufs=1, space="PSUM"))
        psum_conv = ctx.enter_context(
            tc.tile_pool(name="psum_conv", bufs=4, space="PSUM"))

        # ---- zero-init banded early (overlaps weight DMA) ----
        ztile = setup.tile([KR, BL * KER * M_PAD], bf16, tag="ztile")
        nc.vector.memset(ztile[:], 0.0)
        nc.gpsimd.dma_start(
            banded.ap().rearrange("b j r m -> r b j m"),
            ztile[:].rearrange("r (b j m) -> r b j m", b=BL, j=KER),
        )
        # round-robin helper for latency-critical small DMAs: alternate
        # between the two HWDGE rings so the ~0.7us HBM round trips overlap
        _rr = [0]
        def small_dma(dst, src):
            eng = nc.sync if _rr[0] % 2 == 0 else nc.scalar
            _rr[0] += 1
            eng.dma_start(dst, src)

        # ---- FC: kn[n, b] = W[n] . emb[b]  (fc-major output) ----
        # queue plan: both HWDGE rings carry the bulk traffic in priority
        # order (wt -> staging -> band scatter/loads -> dumps); the SWDGE
        # queue only does the banded zero-fill (Q7 desc-gen is too slow
        # for many-descriptor bulk).
        wt_sb = []
        for s in range(WSPLIT):
            w_s = setup.tile([128, CPS * FCOUT], bf16, tag=f"wt{s}")
            eng = nc.sync if s % 2 == 0 else nc.scalar
            eng.dma_start(
                w_s[:], wtp[:, s * CPS * FCOUT:(s + 1) * CPS * FCOUT])
            wt_sb.append(w_s)
        embt_sb = setup.tile([128, NCHUNK * BL], f32, tag="embt")
        nc.scalar.dma_start(embt_sb[:], embtp)
        bb_sb = setup.tile([BL, FCOUT], f32, tag="bb")
        nc.scalar.dma_start(bb_sb[:], bbm)
        id_sb = setup.tile([BL, BL], f32, tag="ident")
        nc.scalar.dma_start(id_sb[:], ident)

        # ---- pre-stage conv rhs for b=0..2 on the SWDGE queue (bulk
        # traffic; keeps both HWDGE rings free for latency-critical DMAs
        # and the scalar engine free for activations) ----
        nslabs = (NCOL + NSLAB - 1) // NSLAB  # 29 (28 full + 1 of 256)
        bounds = [0, (nslabs // 2) * NSLAB, NCOL]
        # staging rides the SWDGE queue: its Q7 descriptor-gen rate is
        # enough to stay ahead of the conv, and it keeps the HWDGE rings'
        # per-engine FIFOs free of bulk so the band chain's completion
        # semaphores aren't stuck behind megabytes of backlog
        staged = {}
        def stage(b):
            parts = []
            for s in range(STG_SPLIT):
                lo, hi = bounds[s], bounds[s + 1]
                p_s = conv_in.tile([KQ, hi - lo], bf16, tag=f"staged{s}")
                nc.gpsimd.dma_start(p_s[:], stg[b, :, lo:hi])
                parts.append(p_s)
            staged[b] = parts
        for b in range(min(3, BL)):
            stage(b)

        # fc matmuls: lhsT = emb chunk (M=4, trivial weight load), rhs =
        # wt chunk (N=225 bf16 stream) -> psum knpT [4, 225]
        embt_bf = setup.tile([128, NCHUNK * BL], bf16, tag="embt_bf")
        nc.vector.tensor_copy(embt_bf[:], embt_sb[:])
        knpT = psum_fc.tile([BL, FCOUT], f32, tag="knpT")
        for ci in range(NCHUNK):
            s, o = divmod(ci, CPS)
            nc.tensor.matmul(
                knpT[:],
                lhsT=embt_bf[:, ci * BL:(ci + 1) * BL],
                rhs=wt_sb[s][:, o * FCOUT:(o + 1) * FCOUT],
                start=(ci == 0), stop=(ci == NCHUNK - 1),
            )

        # knrT = max(fc + (bias - shift), 0) + shift   [4, 225]
        knrT = setup.tile([BL, FCOUT], f32, tag="knrT")
        nc.vector.tensor_tensor(knrT[:], knpT[:], bb_sb[:],
                                op=mybir.AluOpType.add)
        nc.vector.tensor_scalar(knrT[:], knrT[:], 0.0, SHIFT,
                                op0=mybir.AluOpType.max,
                                op1=mybir.AluOpType.add)

        # Z[b, k] = sum_p knrT[b, 25k+p]; zr = 1/Z; rv via PE transpose
        zsum = setup.tile([BL, NK], f32, tag="zsum")
        nc.vector.tensor_reduce(
            zsum[:], knrT[:].rearrange("b (k p) -> b k p", k=NK),
            axis=mybir.AxisListType.X, op=mybir.AluOpType.add)
        zr = setup.tile([BL, NK], f32, tag="zr")
        nc.vector.reciprocal(zr[:], zsum[:])
        zrep = setup.tile([BL, M_FULL], f32, tag="zrep")
        nc.vector.tensor_copy(
            zrep[:].rearrange("b (hh k) -> b hh k", hh=HH),
            zr[:].unsqueeze(1).broadcast_to([BL, HH, NK]),
        )
        zrT = psum_fc.tile([M_FULL, BL], f32, tag="zrT")
        nc.tensor.transpose(zrT[:], zrep[:], id_sb[:])
        rv4 = persist.tile([M_FULL, BL], f32, tag="rv4")
        nc.vector.tensor_copy(rv4[:], zrT[:])
        rv = [rv4[:, b:b + 1] for b in range(BL)]

        # ---- regroup knr to (d j)-partitions ON-CHIP: 9 tiny PE
        # transposes (one per kernel k), no DRAM round trips ----
        # psumT[(d j), k*4 + b] = knrT[b, 25k+5d+j]  (free dim (d j)
        # folds to a single contiguous dim, as the PE weights AP needs)
        psumT = psum_fc.tile([KER * KER, NK * BL], f32, tag="psumT")
        knr_v = knrT[:].rearrange("b (k d j) -> b k d j", k=NK, d=KER)
        for k0 in range(NK):
            nc.tensor.transpose(
                psumT[:, k0 * BL:(k0 + 1) * BL],
                knr_v[:, k0], id_sb[:],
            )
        # kn_k[(d j), (b hh k)] (bf16, hh-replicated)
        kn_k = setup.tile([KER * KER, BL * HH * NK], bf16, tag="kn_k")
        nc.vector.tensor_copy(
            kn_k[:].rearrange("p (b hh k) -> p b hh k", b=BL, hh=HH),
            psumT[:].rearrange("p (k b) -> p b k", k=NK)
            .unsqueeze(2).broadcast_to([KER * KER, BL, HH, NK]),
        )
        # scatter band + immediately load lhsT per sample, so b=0's conv
        # can start while later samples' bands are still being built
        kn_kv = kn_k[:].rearrange("(d j) (b hh k) -> d j b hh k",
                                  j=KER, b=BL, hh=HH)
        lt = []
        for b in range(BL):
            for j in range(KER):
                dst = bass.AP(
                    banded, (b * KER + j) * KR * M_PAD,
                    [[M_PAD, KER],         # d (input-row offset)
                     [M_PAD + NK, HH],     # hh (diagonal: r and m step)
                     [1, NK]],             # k
                )
                src = kn_kv[:, j, b]       # [d(5, stride 5), hh, k]
                small_dma(dst, src)
            lt_b = persist.tile([KQ, M_PAD], bf16, tag=f"lt{b}")
            small_dma(
                lt_b[:],
                banded.ap()[b].rearrange("j r m -> (j r) m"),
            )
            lt.append(lt_b)

        # ---- conv main loop ----
        for b in range(BL):
            parts = staged[b]
            osb = conv_out.tile([M_FULL, NCOL], bf16, tag="osb")
            for mi in range(nslabs):
                o = mi * NSLAB
                n = min(NSLAB, NCOL - o)
                s = 0 if o < bounds[1] else 1
                so = o - bounds[s]
                ps = psum_conv.tile([M_PAD, NSLAB], f32, tag="ps")
                nc.tensor.matmul(
                    ps[:, 0:n], lhsT=lt[b][:],
                    rhs=parts[s][:, so:so + n],
                    start=True, stop=True,
                )
                # evacuate+normalize live rows, alternating DVE / ScalarE
                if mi % 2 == 0:
                    nc.vector.tensor_scalar(
                        osb[:, o:o + n], ps[0:M_FULL, 0:n], rv[b], None,
                        op0=mybir.AluOpType.mult,
                    )
                else:
                    nc.scalar.activation(
                        osb[:, o:o + n], ps[0:M_FULL, 0:n],
                        mybir.ActivationFunctionType.Copy,
                        scale=rv[b],
                    )
            # stage the next sample's rhs now that a conv_in buffer frees
            if b + 3 < BL:
                stage(b + 3)
            # contiguous line-rate dump per sample, split over both rings
            # so the first half drains while the second still evacuates;
            # the tail tile's columns only carry 4 live rows
            vend = (NTILES - 1) * C * W_IMG
            nc.sync.dma_start(out[b, :, 0:bounds[1]],
                              osb[:, 0:bounds[1]])
            nc.scalar.dma_start(out[b, :, bounds[1]:vend],
                                osb[:, bounds[1]:vend])
            nc.scalar.dma_start(out[b, 0:H_LAST * NK, vend:NCOL],
                                osb[0:H_LAST * NK, vend:NCOL])
    nc.compile()
    return nc


def _host_prep(emb, rgb, W, b):
    import ml_dtypes
    bf16 = ml_dtypes.bfloat16

    emb = np.asarray(emb, dtype=np.float32)
    rgb = np.asarray(rgb, dtype=np.float32)
    W = np.asarray(W, dtype=np.float32)
    b = np.asarray(b, dtype=np.float32)

    embt = emb.reshape(B, FCIN).T  # [8192, 32]
    # wtp[p, ci, n] = W[n, ci*128+p]
    wtp = np.ascontiguousarray(
        W.T.reshape(NCHUNK, 128, FCOUT).transpose(1, 0, 2)).astype(bf16)
    wtp = wtp.reshape(128, NCHUNK * FCOUT)
    bbm = np.ascontiguousarray(
        np.broadcast_to((b - SHIFT)[None, :], (BL, FCOUT)).astype(np.float32))
    identm = np.eye(BL, dtype=np.float32)

    # padded rgb, [b, h, c, w] with zero tail rows; bf16
    ph = np.zeros((B, HPAD, C, HP), dtype=bf16)
    ph[:, PAD:PAD + H, :, PAD:PAD + W_IMG] = rgb.transpose(0, 2, 1, 3)
    sb, sh, sc, sw = ph.strides
    stgv = np.lib.stride_tricks.as_strided(
        ph, shape=(B, KER, KR, NTILES, C, W_IMG),
        strides=(sb, sw, sh, HH * sh, sc, sw))
    stg = np.ascontiguousarray(stgv).reshape(B, KQ, NCOL)

    in_maps = []
    for core in range(NCORES):
        sl = slice(core * BL, (core + 1) * BL)
        in_maps.append({
            "embtp": np.ascontiguousarray(
                embt[:, sl].reshape(NCHUNK, 128, BL)
                .transpose(1, 0, 2)).astype(np.float32)
                .reshape(128, NCHUNK * BL),
            "wtp": wtp,
            "bbm": bbm,
            "ident": identm,
            "stg": stg[sl],
        })
    return in_maps


def _unpack(raw):
    """[BL, 126, NCOL] bf16 raw dump -> [BL, 9, 3, 256, 256] f32."""
    a = np.asarray(raw).reshape(BL, HH, NK, NTILES, C, W_IMG)
    a = a.transpose(0, 2, 4, 3, 1, 5)  # [b, k, c, t, hh, w]
    a = a.reshape(BL, NK, C, NTILES * HH, W_IMG)[:, :, :, :H, :]
    return np.ascontiguousarray(a).astype(np.float32)


def get_nc(rep=1):
    key = "nc"
    if key not in _CACHE:
        _CACHE[key] = _build_nc()
    return _CACHE[key]


def kernel(emb, rgb, W, b):
    from concourse.bass_utils import run_bass_kernel_spmd

    assert emb.shape == (B, 128, 8, 8) and rgb.shape == (B, C, H, W_IMG)
    nc = get_nc()
    in_maps = _host_prep(emb, rgb, W, b)
    res = run_bass_kernel_spmd(nc, in_maps, list(range(NCORES)))
    return np.concatenate([_unpack(r["out"]) for r in res.results], axis=0)
